# revision 8
# baseline (speedup 1.0000x reference)
"""KascadeReuseAttention Trainium2 kernel (v2).

Sharding: 16 heads / 8 cores -> 2 heads per core (head/tensor parallel).
Wq/Wk/Wv column-sharded by head, Wo row-sharded; host sums the 8 partial
outputs (the row-parallel all-reduce).

Single SPMD program for all cores: per-core anchor selection enters only as
DATA (per-tile multiplicity weight columns), never as program structure.
Per (head, query-tile t) we compute block attention against ALL past key
tiles v<=t and weight each tile's probabilities by m[h,t,v] = multiplicity
of v in {anchors} + {t} (0 if unselected) before the denominator and PV
matmuls. This reproduces the reference exactly (duplicate anchors included)
while keeping K/V resident in SBUF (no DRAM gather round trip).

v2 changes vs v1 (all aimed at the Tensor engine, the measured bottleneck):
- PV matmul flipped to produce out[q, d] (lhsT=wt, rhs=v_tile), so the
  softmax denominator is a 1-column matmul (rhs=ones) sharing wt as
  stationary, instead of a full 128-column stream per tile pair.
- Normalization uses a per-partition reciprocal [q,1] + tensor_scalar
  multiply; the f32 broadcast matmul is gone. A cheap PE transpose brings
  attn back to [d, s] layout for the output projection.
- Phase-1 q/k matmuls process chunk pairs with the same stationary weight
  tile back-to-back (lets the backend reuse LDWEIGHTS if it can).
- Output partials are written bf16 (halves the output DMA).
- Elementwise work is spread across DVE / Activation / GpSimd.
"""

import math
import sys

import numpy as np

for _p in ("/opt/trn_rl_repo",):
    if _p not in sys.path:
        sys.path.insert(0, _p)

import ml_dtypes  # noqa: E402
import concourse.bass as bass  # noqa: E402
import concourse.mybir as mybir  # noqa: E402
import concourse.tile as tile  # noqa: E402
from concourse.bass_utils import run_bass_kernel_spmd  # noqa: E402
from concourse.vector_clock import ScopedClock  # noqa: E402

BF16 = mybir.dt.bfloat16
F32 = mybir.dt.float32
NPBF16 = ml_dtypes.bfloat16

B, S, E, H, D, K = 1, 4096, 2048, 16, 128, 8
TILE = 128
T = S // TILE          # 32 query/key tiles
NCORES = 8
HPC = H // NCORES      # heads per core = 2
CHUNK = 512            # s-chunk for projections
NCHUNK = S // CHUNK
SM_SCALE = 1.0 / math.sqrt(D)
NTRI = T * (T + 1) // 2  # 528 (t,v<=t) pairs per head

_PATCHED = False


def _patch_tile_drain():
    """This container's walrus caps per-instruction sync waits; the Tile
    kernel-tail drain carries one wait per live semaphore. Split them onto
    preceding SP nops."""
    global _PATCHED
    if _PATCHED:
        return
    _PATCHED = True

    def _drain_and_barrier(self, tick_clock, wait_clock):
        nc = self.nc
        nops = []
        nsems = len(self.sems.allocated()) if self.sems is not None else 0
        for _ in range(nsems):
            nops.append(nc.sync.nop())
        drain_inst = nc.sync.drain()
        wait_clock.add_sem_waits(
            drain_inst.ins, ScopedClock({None: tick_clock.global_clock})
        )
        si = drain_inst.ins.sync_info
        waits = list(si.on_wait or [])
        if len(waits) > 1:
            si.on_wait = waits[:1]
            for i, w in enumerate(waits[1:]):
                ni = nops[i].ins if hasattr(nops[i], "ins") else nops[i]
                nsi = ni.sync_info
                if nsi is None:
                    ni.sync_info = mybir.SyncInfo(on_wait=[w], on_update=[])
                else:
                    nsi.on_wait = [w]
        nc.all_engine_barrier()
        assert self.sems is not None
        popped = nc._tile_sem_poison_stack.pop()
        assert popped is self._sem_poison
        nc.clear_and_free_semaphores(list(self.sems.allocated().values()))
        nc.all_engine_barrier()
        _split_multi_waits(nc)

    tile.TileContext._drain_and_barrier = _drain_and_barrier


def _split_multi_waits(nc):
    """Walrus here encodes at most one sync-wait per instruction; move the
    extras onto preceding same-engine no-ops."""
    ctr = [0]
    for f in nc.m.functions:
        for bb in f.blocks:
            insts = list(bb.instructions)
            if not any(
                i.sync_info and i.sync_info.on_wait
                and len(i.sync_info.on_wait) > 1
                for i in insts
            ):
                continue
            newl = []
            for inst in insts:
                si = inst.sync_info
                if si and si.on_wait and len(si.on_wait) > 1:
                    waits = list(si.on_wait)
                    for w in waits[:-1]:
                        ctr[0] += 1
                        nop = mybir.InstNoOp(
                            name=f"WSPL-{ctr[0]}", ins=[], outs=[])
                        nop.engine = inst.engine
                        nop.sync_info = mybir.SyncInfo(
                            on_wait=[w], on_update=[])
                        newl.append(nop)
                    si.on_wait = waits[-1:]
                newl.append(inst)
            bb.instructions = newl


def _tri_col(t, v):
    return t * (t + 1) // 2 + v


def build_bass():
    """Uniform per-core program. Inputs (per core, bf16 unless noted):
    xT [E, S], wqk [E, 4*128] (q_h0,q_h1,k_h0,k_h1), wv [E, 256],
    wo [256, E], cosT/sinT [128, S], rotT [128,128] (R^T for rotate_half),
    triT [128,128] (tri[l,q] = l<=q), idT [128,128] identity,
    mw [128, HPC*NTRI] f32 weight columns.
    Output: outT [E, S] bf16 (partial x@.. contribution of this core's heads).
    """
    nc = bass.Bass()
    xT = nc.dram_tensor("xT", [E, S], BF16, kind="ExternalInput")
    wqk = nc.dram_tensor("wqk", [E, 4 * TILE], BF16, kind="ExternalInput")
    wv = nc.dram_tensor("wv", [E, 2 * TILE], BF16, kind="ExternalInput")
    wo = nc.dram_tensor("wo", [2 * TILE, E], BF16, kind="ExternalInput")
    cosT = nc.dram_tensor("cosT", [TILE, S], BF16, kind="ExternalInput")
    sinT = nc.dram_tensor("sinT", [TILE, S], BF16, kind="ExternalInput")
    rotT = nc.dram_tensor("rotT", [TILE, TILE], BF16, kind="ExternalInput")
    triT = nc.dram_tensor("triT", [TILE, TILE], BF16, kind="ExternalInput")
    idT = nc.dram_tensor("idT", [TILE, TILE], BF16, kind="ExternalInput")
    mw = nc.dram_tensor("mw", [TILE, HPC * NTRI], F32, kind="ExternalInput")
    outT = nc.dram_tensor("outT", [E, S], BF16, kind="ExternalOutput")

    EK = E // TILE  # 16 contraction tiles

    with tile.TileContext(nc) as tc:
        with tc.tile_pool(name="const", bufs=1) as cpool:
            sb_wqk = cpool.tile([TILE, EK, 4 * TILE], BF16)
            sb_wv = cpool.tile([TILE, EK, 2 * TILE], BF16)
            sb_wo = cpool.tile([TILE, 2, E], BF16)
            sb_cos = cpool.tile([TILE, S], BF16)
            sb_sin = cpool.tile([TILE, S], BF16)
            sb_rot = cpool.tile([TILE, TILE], BF16)
            sb_tri = cpool.tile([TILE, TILE], BF16)
            sb_id = cpool.tile([TILE, TILE], BF16)
            sb_mw = cpool.tile([TILE, HPC * NTRI], F32)
            ones_col = cpool.tile([TILE, 1], BF16)
            # persistent per-head tensors (bf16): qT/kT [d, S], v [s-tiles, d]
            sb_q = cpool.tile([TILE, HPC, S], BF16, tag="q")
            sb_k = cpool.tile([TILE, HPC, S], BF16, tag="k")
            sb_v = cpool.tile([TILE, HPC, S], BF16, tag="v")
            sb_attn = cpool.tile([TILE, HPC, S], BF16, tag="attn")

            nc.sync.dma_start(out=sb_wqk[:],
                              in_=wqk.rearrange("(a p) b -> p a b", p=TILE))
            nc.sync.dma_start(out=sb_wv[:],
                              in_=wv.rearrange("(a p) b -> p a b", p=TILE))
            nc.sync.dma_start(out=sb_wo[:],
                              in_=wo.rearrange("(a p) b -> p a b", p=TILE))
            nc.sync.dma_start(out=sb_cos[:], in_=cosT[:])
            nc.sync.dma_start(out=sb_sin[:], in_=sinT[:])
            nc.sync.dma_start(out=sb_rot[:], in_=rotT[:])
            nc.sync.dma_start(out=sb_tri[:], in_=triT[:])
            nc.sync.dma_start(out=sb_id[:], in_=idT[:])
            nc.sync.dma_start(out=sb_mw[:], in_=mw[:])
            nc.vector.memset(ones_col[:], 1.0)

            # ---------------- Phase 1: projections + RoPE ----------------
            # Chunk pairs: for each m-tile the two chunks' accumulation
            # steps share the stationary weight tile back-to-back.
            with (
                tc.tile_pool(name="xin", bufs=2) as xpool,
                tc.tile_pool(name="ptmp", bufs=3) as tpool,
                tc.tile_pool(name="qkps", bufs=2, space="PSUM") as qkps,
                tc.tile_pool(name="vps", bufs=2, space="PSUM") as vps,
                tc.tile_pool(name="rops", bufs=2, space="PSUM") as rops,
            ):
                for cp in range(NCHUNK // 2):
                    xts = []
                    for half in range(2):
                        s0 = (2 * cp + half) * CHUNK
                        xt = xpool.tile([TILE, EK, CHUNK], BF16, tag="xt")
                        nc.sync.dma_start(
                            out=xt[:],
                            in_=xT[:, s0:s0 + CHUNK].rearrange(
                                "(a p) b -> p a b", p=TILE),
                        )
                        xts.append(xt)
                    # qT/kT M-tiles: 0=q_h0 1=q_h1 2=k_h0 3=k_h1
                    for m in range(4):
                        pss = [qkps.tile([TILE, CHUNK], F32, tag="qk",
                                         name=f"qk{m}a"),
                               qkps.tile([TILE, CHUNK], F32, tag="qk",
                                         name=f"qk{m}b")]
                        for e in range(EK):
                            for half in range(2):
                                nc.tensor.matmul(
                                    pss[half][:],
                                    sb_wqk[:, e, m * TILE:(m + 1) * TILE],
                                    xts[half][:, e, :], start=(e == 0),
                                    stop=(e == EK - 1),
                                    skip_group_check=True)
                        for half in range(2):
                            s0 = (2 * cp + half) * CHUNK
                            ps = pss[half]
                            raw = tpool.tile([TILE, CHUNK], BF16, tag="raw")
                            nc.scalar.copy(out=raw[:], in_=ps[:])
                            rot = rops.tile([TILE, CHUNK], F32, tag="rot")
                            nc.tensor.matmul(rot[:], sb_rot[:], raw[:],
                                             start=True, stop=True)
                            t1 = tpool.tile([TILE, CHUNK], BF16, tag="t1")
                            nc.gpsimd.tensor_mul(t1[:], raw[:],
                                                 sb_cos[:, s0:s0 + CHUNK])
                            t2 = tpool.tile([TILE, CHUNK], BF16, tag="t2")
                            nc.vector.tensor_mul(t2[:], rot[:],
                                                 sb_sin[:, s0:s0 + CHUNK])
                            dst = sb_q if m < 2 else sb_k
                            h = m % 2
                            nc.vector.tensor_add(dst[:, h, s0:s0 + CHUNK],
                                                 t1[:], t2[:])
                    # v: M-tiles over s (8 per chunk pair), N = 2 heads * 128
                    for sm in range(2 * CHUNK // TILE):
                        vp = vps.tile([TILE, 2 * TILE], F32, tag="v")
                        st = 2 * cp * CHUNK + sm * TILE
                        xt = xts[sm // (CHUNK // TILE)]
                        lo = (sm % (CHUNK // TILE)) * TILE
                        for e in range(EK):
                            nc.tensor.matmul(
                                vp[:], xt[:, e, lo:lo + TILE],
                                sb_wv[:, e, :], start=(e == 0),
                                stop=(e == EK - 1))
                        for h in range(HPC):
                            if h == 0:
                                nc.scalar.copy(
                                    out=sb_v[:, h, st:st + TILE],
                                    in_=vp[:, h * TILE:(h + 1) * TILE])
                            else:
                                nc.vector.tensor_copy(
                                    sb_v[:, h, st:st + TILE],
                                    vp[:, h * TILE:(h + 1) * TILE])

            # ---------------- Phase 2: block-sparse attention ------------
            # Per (h, t): logits [l,q] per past tile v, exp, multiplicity
            # weight, then PV flipped to out[q, d] with wt stationary so the
            # denominator is a 1-column matmul. Normalize with a [q,1]
            # reciprocal, transpose back to [d, q] on the PE.
            GRP = 4  # logits tiles per psum bank
            with (
                tc.tile_pool(name="wt", bufs=3) as wtp,
                tc.tile_pool(name="nrm", bufs=3) as nrm,
                tc.tile_pool(name="lg", bufs=2, space="PSUM") as lgps,
                tc.tile_pool(name="ot", bufs=2, space="PSUM") as otps,
                tc.tile_pool(name="dn", bufs=2, space="PSUM") as dnps,
                tc.tile_pool(name="tp", bufs=2, space="PSUM") as tpps,
            ):
                alt = 0
                for h in range(HPC):
                    for t in range(T):
                        nv = t + 1
                        q_sl = sb_q[:, h, t * TILE:(t + 1) * TILE]
                        out_ps = otps.tile([TILE, TILE], F32, tag="ot")
                        den_ps = dnps.tile([TILE, 1], F32, tag="dn")
                        ngrp = (nv + GRP - 1) // GRP
                        first = True
                        for g in range(ngrp):
                            v0 = g * GRP
                            gn = min(GRP, nv - v0)
                            lg = lgps.tile([TILE, GRP * TILE], F32, tag="lg")
                            for j in range(gn):
                                v = v0 + j
                                nc.tensor.matmul(
                                    lg[:, j * TILE:(j + 1) * TILE],
                                    sb_k[:, h, v * TILE:(v + 1) * TILE],
                                    q_sl, start=True, stop=True)
                            wt = wtp.tile([TILE, GRP * TILE], BF16, tag="wt")
                            nc.scalar.activation(
                                out=wt[:, :gn * TILE], in_=lg[:, :gn * TILE],
                                func=mybir.ActivationFunctionType.Exp,
                                scale=SM_SCALE)
                            for j in range(gn):
                                v = v0 + j
                                mcol = sb_mw[:, h * NTRI + _tri_col(t, v):
                                             h * NTRI + _tri_col(t, v) + 1]
                                wsl = wt[:, j * TILE:(j + 1) * TILE]
                                if v == t:
                                    nc.vector.scalar_tensor_tensor(
                                        out=wsl, in0=wsl, scalar=mcol,
                                        in1=sb_tri[:],
                                        op0=mybir.AluOpType.mult,
                                        op1=mybir.AluOpType.mult)
                                else:
                                    eng = nc.vector if alt % 2 == 0 \
                                        else nc.gpsimd
                                    eng.tensor_scalar_mul(wsl, wsl, mcol)
                                    alt += 1
                            for j in range(gn):
                                v = v0 + j
                                last = (g == ngrp - 1) and (j == gn - 1)
                                wsl = wt[:, j * TILE:(j + 1) * TILE]
                                # out[q, d] += wt^T-contracted v tile;
                                # den[q, 1] shares wt as stationary.
                                nc.tensor.matmul(
                                    out_ps[:], wsl,
                                    sb_v[:, h, v * TILE:(v + 1) * TILE],
                                    start=first, stop=last,
                                    skip_group_check=True)
                                nc.tensor.matmul(
                                    den_ps[:], wsl, ones_col[:],
                                    start=first, stop=last,
                                    skip_group_check=True)
                                first = False
                        # normalize: per-partition recip, scale, transpose
                        rc = nrm.tile([TILE, 1], F32, tag="rc")
                        nc.vector.reciprocal(out=rc[:], in_=den_ps[:])
                        at = nrm.tile([TILE, TILE], BF16, tag="at")
                        nc.vector.tensor_scalar_mul(at[:], out_ps[:], rc[:])
                        tp = tpps.tile([TILE, TILE], BF16, tag="tp")
                        nc.tensor.transpose(tp[:], at[:], sb_id[:])
                        nc.scalar.copy(
                            out=sb_attn[:, h, t * TILE:(t + 1) * TILE],
                            in_=tp[:])

            # ---------------- Phase 3: output projection -----------------
            with (
                tc.tile_pool(name="ost", bufs=3) as ost,
                tc.tile_pool(name="wops", bufs=3, space="PSUM") as wops,
            ):
                for ci in range(NCHUNK):
                    s0 = ci * CHUNK
                    for m in range(EK):  # output e tiles
                        op = wops.tile([TILE, CHUNK], F32, tag="op")
                        for h in range(HPC):
                            nc.tensor.matmul(
                                op[:], sb_wo[:, h, m * TILE:(m + 1) * TILE],
                                sb_attn[:, h, s0:s0 + CHUNK],
                                start=(h == 0), stop=(h == HPC - 1))
                        ob = ost.tile([TILE, CHUNK], BF16, tag="ob")
                        if m % 2 == 0:
                            nc.scalar.copy(out=ob[:], in_=op[:])
                        else:
                            nc.vector.tensor_copy(ob[:], op[:])
                        nc.sync.dma_start(
                            out=outT[m * TILE:(m + 1) * TILE, s0:s0 + CHUNK],
                            in_=ob[:])
    return nc


def _host_prep(x, wq, wk, wv, wo, rope_angles, anchor_indices):
    xT = np.ascontiguousarray(x[0].T).astype(NPBF16)
    cos = np.cos(rope_angles.astype(np.float64))
    sin = np.sin(rope_angles.astype(np.float64))
    cosT = np.ascontiguousarray(
        np.concatenate([cos, cos], axis=1).T).astype(NPBF16)
    sinT = np.ascontiguousarray(
        np.concatenate([sin, sin], axis=1).T).astype(NPBF16)
    half = D // 2
    R = np.zeros((D, D), np.float32)
    for d in range(half):
        R[d, d + half] = -1.0
        R[d + half, d] = 1.0
    rotT = np.ascontiguousarray(R.T).astype(NPBF16)
    tri = (np.arange(TILE)[:, None] <= np.arange(TILE)[None, :])
    triT = tri.astype(NPBF16)
    idT = np.eye(TILE, dtype=NPBF16)

    in_maps = []
    for c in range(NCORES):
        heads = [c * HPC + i for i in range(HPC)]
        wqk_c = np.concatenate(
            [wq[:, h * D:(h + 1) * D] for h in heads]
            + [wk[:, h * D:(h + 1) * D] for h in heads], axis=1)
        wv_c = np.concatenate([wv[:, h * D:(h + 1) * D] for h in heads],
                              axis=1)
        wo_c = np.concatenate([wo[h * D:(h + 1) * D, :] for h in heads],
                              axis=0)
        mwc = np.zeros((TILE, HPC * NTRI), np.float32)
        for i, h in enumerate(heads):
            for t in range(T):
                sel = list(anchor_indices[0, h, t]) + [t]
                for v in range(t + 1):
                    m = sel.count(v)
                    if m:
                        mwc[:, i * NTRI + _tri_col(t, v)] = float(m)
        in_maps.append({
            "xT": xT, "wqk": np.ascontiguousarray(wqk_c).astype(NPBF16),
            "wv": np.ascontiguousarray(wv_c).astype(NPBF16),
            "wo": np.ascontiguousarray(wo_c).astype(NPBF16),
            "cosT": cosT, "sinT": sinT, "rotT": rotT, "triT": triT,
            "idT": idT, "mw": mwc,
        })
    return in_maps


def kernel(x, wq, wk, wv, wo, rope_angles, anchor_indices, **run_kwargs):
    _patch_tile_drain()
    nc = build_bass()
    in_maps = _host_prep(x, wq, wk, wv, wo, rope_angles, anchor_indices)
    res = run_bass_kernel_spmd(nc, in_maps, core_ids=list(range(NCORES)),
                               **run_kwargs)
    acc = np.zeros((E, S), np.float64)
    for c in range(NCORES):
        acc += res.results[c]["outT"].astype(np.float64)
    out = np.ascontiguousarray(acc.T.reshape(B, S, E)).astype(np.float32)
    kernel.last_results = res
    return out


# revision 11
# speedup vs baseline: 1.1823x; 1.1823x over previous
"""KascadeReuseAttention Trainium2 kernel (v2).

Sharding: 16 heads / 8 cores -> 2 heads per core (head/tensor parallel).
Wq/Wk/Wv column-sharded by head, Wo row-sharded; host sums the 8 partial
outputs (the row-parallel all-reduce).

Single SPMD program for all cores: per-core anchor selection enters only as
DATA (per-tile multiplicity weight columns), never as program structure.
Per (head, query-tile t) we compute block attention against ALL past key
tiles v<=t and weight each tile's probabilities by m[h,t,v] = multiplicity
of v in {anchors} + {t} (0 if unselected) before the denominator and PV
matmuls. This reproduces the reference exactly (duplicate anchors included)
while keeping K/V resident in SBUF (no DRAM gather round trip).

v2 changes vs v1 (all aimed at the Tensor engine, the measured bottleneck):
- PV matmul flipped to produce out[q, d] (lhsT=wt, rhs=v_tile), so the
  softmax denominator is a 1-column matmul (rhs=ones) sharing wt as
  stationary, instead of a full 128-column stream per tile pair.
- Normalization uses a per-partition reciprocal [q,1] + tensor_scalar
  multiply; the f32 broadcast matmul is gone. A cheap PE transpose brings
  attn back to [d, s] layout for the output projection.
- Phase-1 q/k matmuls process chunk pairs with the same stationary weight
  tile back-to-back (lets the backend reuse LDWEIGHTS if it can).
- Output partials are written bf16 (halves the output DMA).
- Elementwise work is spread across DVE / Activation / GpSimd.
"""

import math
import sys

import numpy as np

for _p in ("/opt/trn_rl_repo",):
    if _p not in sys.path:
        sys.path.insert(0, _p)

import ml_dtypes  # noqa: E402
import concourse.bass as bass  # noqa: E402
import concourse.mybir as mybir  # noqa: E402
import concourse.tile as tile  # noqa: E402
from concourse.bass_utils import run_bass_kernel_spmd  # noqa: E402
from concourse.vector_clock import ScopedClock  # noqa: E402

BF16 = mybir.dt.bfloat16
F32 = mybir.dt.float32
NPBF16 = ml_dtypes.bfloat16

B, S, E, H, D, K = 1, 4096, 2048, 16, 128, 8
TILE = 128
T = S // TILE          # 32 query/key tiles
NCORES = 8
HPC = H // NCORES      # heads per core = 2
CHUNK = 512            # s-chunk for projections
NCHUNK = S // CHUNK
SM_SCALE = 1.0 / math.sqrt(D)
NTRI = T * (T + 1) // 2  # 528 (t,v<=t) pairs per head

_PATCHED = False


def _patch_tile_drain():
    """This container's walrus caps per-instruction sync waits; the Tile
    kernel-tail drain carries one wait per live semaphore. Split them onto
    preceding SP nops."""
    global _PATCHED
    if _PATCHED:
        return
    _PATCHED = True

    def _drain_and_barrier(self, tick_clock, wait_clock):
        nc = self.nc
        nops = []
        nsems = len(self.sems.allocated()) if self.sems is not None else 0
        for _ in range(nsems):
            nops.append(nc.sync.nop())
        drain_inst = nc.sync.drain()
        wait_clock.add_sem_waits(
            drain_inst.ins, ScopedClock({None: tick_clock.global_clock})
        )
        si = drain_inst.ins.sync_info
        waits = list(si.on_wait or [])
        if len(waits) > 1:
            si.on_wait = waits[:1]
            for i, w in enumerate(waits[1:]):
                ni = nops[i].ins if hasattr(nops[i], "ins") else nops[i]
                nsi = ni.sync_info
                if nsi is None:
                    ni.sync_info = mybir.SyncInfo(on_wait=[w], on_update=[])
                else:
                    nsi.on_wait = [w]
        nc.all_engine_barrier()
        assert self.sems is not None
        popped = nc._tile_sem_poison_stack.pop()
        assert popped is self._sem_poison
        nc.clear_and_free_semaphores(list(self.sems.allocated().values()))
        nc.all_engine_barrier()
        _split_multi_waits(nc)

    tile.TileContext._drain_and_barrier = _drain_and_barrier


def _split_multi_waits(nc):
    """Walrus here encodes at most one sync-wait per instruction; move the
    extras onto preceding same-engine no-ops."""
    ctr = [0]
    for f in nc.m.functions:
        for bb in f.blocks:
            insts = list(bb.instructions)
            if not any(
                i.sync_info and i.sync_info.on_wait
                and len(i.sync_info.on_wait) > 1
                for i in insts
            ):
                continue
            newl = []
            for inst in insts:
                si = inst.sync_info
                if si and si.on_wait and len(si.on_wait) > 1:
                    waits = list(si.on_wait)
                    for w in waits[:-1]:
                        ctr[0] += 1
                        nop = mybir.InstNoOp(
                            name=f"WSPL-{ctr[0]}", ins=[], outs=[])
                        nop.engine = inst.engine
                        nop.sync_info = mybir.SyncInfo(
                            on_wait=[w], on_update=[])
                        newl.append(nop)
                    si.on_wait = waits[-1:]
                newl.append(inst)
            bb.instructions = newl


def _tri_col(t, v):
    return t * (t + 1) // 2 + v


def build_bass():
    """Uniform per-core program. Inputs (per core, bf16 unless noted):
    xT [E, S], wqk [E, 4*128] (q_h0,q_h1,k_h0,k_h1), wv [E, 256],
    wo [256, E], cosT/sinT [128, S], rotT [128,128] (R^T for rotate_half),
    triT [128,128] (tri[l,q] = l<=q), idT [128,128] identity,
    mw [128, HPC*NTRI] f32 weight columns.
    Output: outT [E, S] bf16 (partial x@.. contribution of this core's heads).
    """
    nc = bass.Bass()
    xT = nc.dram_tensor("xT", [E, S], BF16, kind="ExternalInput")
    wqk = nc.dram_tensor("wqk", [E, 4 * TILE], BF16, kind="ExternalInput")
    wv = nc.dram_tensor("wv", [E, 2 * TILE], BF16, kind="ExternalInput")
    wo = nc.dram_tensor("wo", [2 * TILE, E], BF16, kind="ExternalInput")
    cosT = nc.dram_tensor("cosT", [TILE, S], BF16, kind="ExternalInput")
    sinT = nc.dram_tensor("sinT", [TILE, S], BF16, kind="ExternalInput")
    rotT = nc.dram_tensor("rotT", [TILE, TILE], BF16, kind="ExternalInput")
    triT = nc.dram_tensor("triT", [TILE, TILE], BF16, kind="ExternalInput")
    idT = nc.dram_tensor("idT", [TILE, TILE], BF16, kind="ExternalInput")
    mw = nc.dram_tensor("mw", [TILE, HPC * NTRI], F32, kind="ExternalInput")
    outT = nc.dram_tensor("outT", [E, S], BF16, kind="ExternalOutput")

    EK = E // TILE  # 16 contraction tiles

    with tile.TileContext(nc) as tc:
        with tc.tile_pool(name="const", bufs=1) as cpool:
            sb_wqk = cpool.tile([TILE, EK, 4 * TILE], BF16)
            sb_wv = cpool.tile([TILE, EK, 2 * TILE], BF16)
            sb_wo = cpool.tile([TILE, 2, E], BF16)
            sb_cos = cpool.tile([TILE, S], BF16)
            sb_sin = cpool.tile([TILE, S], BF16)
            sb_rot = cpool.tile([TILE, TILE], BF16)
            sb_tri = cpool.tile([TILE, TILE], BF16)
            sb_id = cpool.tile([TILE, TILE], BF16)
            sb_mw = cpool.tile([TILE, HPC * NTRI], F32)
            ones_col = cpool.tile([TILE, 1], BF16)
            # persistent per-head tensors (bf16): qT/kT [d, S], v [s-tiles, d]
            sb_q = cpool.tile([TILE, HPC, S], BF16, tag="q")
            sb_k = cpool.tile([TILE, HPC, S], BF16, tag="k")
            sb_v = cpool.tile([TILE, HPC, S], BF16, tag="v")
            sb_attn = cpool.tile([TILE, HPC, S], BF16, tag="attn")

            nc.sync.dma_start(out=sb_wqk[:],
                              in_=wqk.rearrange("(a p) b -> p a b", p=TILE))
            nc.sync.dma_start(out=sb_wv[:],
                              in_=wv.rearrange("(a p) b -> p a b", p=TILE))
            nc.sync.dma_start(out=sb_wo[:],
                              in_=wo.rearrange("(a p) b -> p a b", p=TILE))
            nc.sync.dma_start(out=sb_cos[:], in_=cosT[:])
            nc.sync.dma_start(out=sb_sin[:], in_=sinT[:])
            nc.sync.dma_start(out=sb_rot[:], in_=rotT[:])
            nc.sync.dma_start(out=sb_tri[:], in_=triT[:])
            nc.sync.dma_start(out=sb_id[:], in_=idT[:])
            nc.sync.dma_start(out=sb_mw[:], in_=mw[:])
            nc.vector.memset(ones_col[:], 1.0)

            # ---------------- Phase 1: projections + RoPE ----------------
            # Chunk pairs: for each m-tile the two chunks' accumulation
            # steps share the stationary weight tile back-to-back.
            with (
                tc.tile_pool(name="xin", bufs=2) as xpool,
                tc.tile_pool(name="ptmp", bufs=3) as tpool,
                tc.tile_pool(name="qkps", bufs=2, space="PSUM") as qkps,
                tc.tile_pool(name="vps", bufs=2, space="PSUM") as vps,
                tc.tile_pool(name="rops", bufs=2, space="PSUM") as rops,
            ):
                for cp in range(NCHUNK // 2):
                    xts = []
                    for half in range(2):
                        s0 = (2 * cp + half) * CHUNK
                        xt = xpool.tile([TILE, EK, CHUNK], BF16, tag="xt")
                        nc.sync.dma_start(
                            out=xt[:],
                            in_=xT[:, s0:s0 + CHUNK].rearrange(
                                "(a p) b -> p a b", p=TILE),
                        )
                        xts.append(xt)
                    # qT/kT M-tiles: 0=q_h0 1=q_h1 2=k_h0 3=k_h1
                    for m in range(4):
                        pss = [qkps.tile([TILE, CHUNK], F32, tag="qk",
                                         name=f"qk{m}a"),
                               qkps.tile([TILE, CHUNK], F32, tag="qk",
                                         name=f"qk{m}b")]
                        for e in range(EK):
                            for half in range(2):
                                nc.tensor.matmul(
                                    pss[half][:],
                                    sb_wqk[:, e, m * TILE:(m + 1) * TILE],
                                    xts[half][:, e, :], start=(e == 0),
                                    stop=(e == EK - 1),
                                    skip_group_check=True)
                        for half in range(2):
                            s0 = (2 * cp + half) * CHUNK
                            ps = pss[half]
                            raw = tpool.tile([TILE, CHUNK], BF16, tag="raw")
                            nc.scalar.copy(out=raw[:], in_=ps[:])
                            rot = rops.tile([TILE, CHUNK], F32, tag="rot")
                            nc.tensor.matmul(rot[:], sb_rot[:], raw[:],
                                             start=True, stop=True)
                            t1 = tpool.tile([TILE, CHUNK], BF16, tag="t1")
                            nc.gpsimd.tensor_mul(t1[:], raw[:],
                                                 sb_cos[:, s0:s0 + CHUNK])
                            t2 = tpool.tile([TILE, CHUNK], BF16, tag="t2")
                            nc.vector.tensor_mul(t2[:], rot[:],
                                                 sb_sin[:, s0:s0 + CHUNK])
                            dst = sb_q if m < 2 else sb_k
                            h = m % 2
                            nc.vector.tensor_add(dst[:, h, s0:s0 + CHUNK],
                                                 t1[:], t2[:])
                    # v: M-tiles over s (8 per chunk pair), N = 2 heads * 128
                    for sm in range(2 * CHUNK // TILE):
                        vp = vps.tile([TILE, 2 * TILE], F32, tag="v")
                        st = 2 * cp * CHUNK + sm * TILE
                        xt = xts[sm // (CHUNK // TILE)]
                        lo = (sm % (CHUNK // TILE)) * TILE
                        for e in range(EK):
                            nc.tensor.matmul(
                                vp[:], xt[:, e, lo:lo + TILE],
                                sb_wv[:, e, :], start=(e == 0),
                                stop=(e == EK - 1))
                        for h in range(HPC):
                            if h == 0:
                                nc.scalar.copy(
                                    out=sb_v[:, h, st:st + TILE],
                                    in_=vp[:, h * TILE:(h + 1) * TILE])
                            else:
                                nc.vector.tensor_copy(
                                    sb_v[:, h, st:st + TILE],
                                    vp[:, h * TILE:(h + 1) * TILE])

            # ---------------- Phase 2: block-sparse attention ------------
            # Per (h, t): logits [l,q] per past tile v, exp, multiplicity
            # weight, then PV flipped to out[q, d] with wt stationary so the
            # denominator is a 1-column matmul. Normalize with a [q,1]
            # reciprocal, transpose back to [d, q] on the PE.
            GRP = 4  # logits tiles per psum bank
            with (
                tc.tile_pool(name="wt", bufs=3) as wtp,
                tc.tile_pool(name="nrm", bufs=3) as nrm,
                tc.tile_pool(name="lg", bufs=2, space="PSUM") as lgps,
                tc.tile_pool(name="ot", bufs=2, space="PSUM") as otps,
                tc.tile_pool(name="dn", bufs=2, space="PSUM") as dnps,
                tc.tile_pool(name="tp", bufs=2, space="PSUM") as tpps,
            ):
                pending = None  # (at_tile, h, t) awaiting transpose
                for h in range(HPC):
                    for t in range(T):
                        nv = t + 1
                        q_sl = sb_q[:, h, t * TILE:(t + 1) * TILE]
                        out_ps = otps.tile([TILE, TILE], F32, tag="ot")
                        den_ps = dnps.tile([TILE, 1], F32, tag="dn")
                        ngrp = (nv + GRP - 1) // GRP
                        first = True
                        for g in range(ngrp):
                            v0 = g * GRP
                            gn = min(GRP, nv - v0)
                            lg = lgps.tile([TILE, GRP * TILE], F32, tag="lg")
                            for j in range(gn):
                                v = v0 + j
                                nc.tensor.matmul(
                                    lg[:, j * TILE:(j + 1) * TILE],
                                    sb_k[:, h, v * TILE:(v + 1) * TILE],
                                    q_sl, start=True, stop=True)
                            wt = wtp.tile([TILE, GRP * TILE], BF16, tag="wt")
                            nc.scalar.activation(
                                out=wt[:, :gn * TILE], in_=lg[:, :gn * TILE],
                                func=mybir.ActivationFunctionType.Exp,
                                scale=SM_SCALE)
                            for j in range(gn):
                                v = v0 + j
                                mcol = sb_mw[:, h * NTRI + _tri_col(t, v):
                                             h * NTRI + _tri_col(t, v) + 1]
                                wsl = wt[:, j * TILE:(j + 1) * TILE]
                                if v == t:
                                    nc.vector.scalar_tensor_tensor(
                                        out=wsl, in0=wsl, scalar=mcol,
                                        in1=sb_tri[:],
                                        op0=mybir.AluOpType.mult,
                                        op1=mybir.AluOpType.mult)
                                else:
                                    nc.vector.tensor_scalar_mul(wsl, wsl,
                                                                mcol)
                            for j in range(gn):
                                v = v0 + j
                                last = (g == ngrp - 1) and (j == gn - 1)
                                wsl = wt[:, j * TILE:(j + 1) * TILE]
                                # out[q, d] += wt^T-contracted v tile;
                                # den[q, 1] shares wt as stationary.
                                nc.tensor.matmul(
                                    out_ps[:], wsl,
                                    sb_v[:, h, v * TILE:(v + 1) * TILE],
                                    start=first, stop=last,
                                    skip_group_check=True)
                                nc.tensor.matmul(
                                    den_ps[:], wsl, ones_col[:],
                                    start=first, stop=last,
                                    skip_group_check=True)
                                first = False
                        # normalize: per-partition recip (DVE), scale on ACT
                        rc = nrm.tile([TILE, 1], F32, tag="rc")
                        nc.vector.reciprocal(out=rc[:], in_=den_ps[:])
                        at = nrm.tile([TILE, TILE], BF16, tag="at")
                        nc.scalar.activation(
                            out=at[:], in_=out_ps[:],
                            func=mybir.ActivationFunctionType.Copy,
                            scale=rc[:])
                        # transpose back to [d, q] one iteration late so the
                        # PE never stalls on the recip/normalize chain
                        if pending is not None:
                            pat, ph, pt = pending
                            tp = tpps.tile([TILE, TILE], BF16, tag="tp")
                            nc.tensor.transpose(tp[:], pat[:], sb_id[:])
                            nc.scalar.copy(
                                out=sb_attn[:, ph, pt * TILE:(pt + 1) * TILE],
                                in_=tp[:])
                        pending = (at, h, t)
                if pending is not None:
                    pat, ph, pt = pending
                    tp = tpps.tile([TILE, TILE], BF16, tag="tp")
                    nc.tensor.transpose(tp[:], pat[:], sb_id[:])
                    nc.scalar.copy(
                        out=sb_attn[:, ph, pt * TILE:(pt + 1) * TILE],
                        in_=tp[:])

            # ---------------- Phase 3: output projection -----------------
            with (
                tc.tile_pool(name="ost", bufs=3) as ost,
                tc.tile_pool(name="wops", bufs=3, space="PSUM") as wops,
            ):
                for ci in range(NCHUNK):
                    s0 = ci * CHUNK
                    for m in range(EK):  # output e tiles
                        op = wops.tile([TILE, CHUNK], F32, tag="op")
                        for h in range(HPC):
                            nc.tensor.matmul(
                                op[:], sb_wo[:, h, m * TILE:(m + 1) * TILE],
                                sb_attn[:, h, s0:s0 + CHUNK],
                                start=(h == 0), stop=(h == HPC - 1))
                        ob = ost.tile([TILE, CHUNK], BF16, tag="ob")
                        if m % 2 == 0:
                            nc.scalar.copy(out=ob[:], in_=op[:])
                        else:
                            nc.vector.tensor_copy(ob[:], op[:])
                        nc.sync.dma_start(
                            out=outT[m * TILE:(m + 1) * TILE, s0:s0 + CHUNK],
                            in_=ob[:])
    return nc


def _host_prep(x, wq, wk, wv, wo, rope_angles, anchor_indices):
    xT = np.ascontiguousarray(x[0].T).astype(NPBF16)
    cos = np.cos(rope_angles.astype(np.float64))
    sin = np.sin(rope_angles.astype(np.float64))
    cosT = np.ascontiguousarray(
        np.concatenate([cos, cos], axis=1).T).astype(NPBF16)
    sinT = np.ascontiguousarray(
        np.concatenate([sin, sin], axis=1).T).astype(NPBF16)
    half = D // 2
    R = np.zeros((D, D), np.float32)
    for d in range(half):
        R[d, d + half] = -1.0
        R[d + half, d] = 1.0
    rotT = np.ascontiguousarray(R.T).astype(NPBF16)
    tri = (np.arange(TILE)[:, None] <= np.arange(TILE)[None, :])
    triT = tri.astype(NPBF16)
    idT = np.eye(TILE, dtype=NPBF16)

    in_maps = []
    for c in range(NCORES):
        heads = [c * HPC + i for i in range(HPC)]
        wqk_c = np.concatenate(
            [wq[:, h * D:(h + 1) * D] for h in heads]
            + [wk[:, h * D:(h + 1) * D] for h in heads], axis=1)
        wv_c = np.concatenate([wv[:, h * D:(h + 1) * D] for h in heads],
                              axis=1)
        wo_c = np.concatenate([wo[h * D:(h + 1) * D, :] for h in heads],
                              axis=0)
        mwc = np.zeros((TILE, HPC * NTRI), np.float32)
        for i, h in enumerate(heads):
            for t in range(T):
                sel = list(anchor_indices[0, h, t]) + [t]
                for v in range(t + 1):
                    m = sel.count(v)
                    if m:
                        mwc[:, i * NTRI + _tri_col(t, v)] = float(m)
        in_maps.append({
            "xT": xT, "wqk": np.ascontiguousarray(wqk_c).astype(NPBF16),
            "wv": np.ascontiguousarray(wv_c).astype(NPBF16),
            "wo": np.ascontiguousarray(wo_c).astype(NPBF16),
            "cosT": cosT, "sinT": sinT, "rotT": rotT, "triT": triT,
            "idT": idT, "mw": mwc,
        })
    return in_maps


def kernel(x, wq, wk, wv, wo, rope_angles, anchor_indices, **run_kwargs):
    _patch_tile_drain()
    nc = build_bass()
    in_maps = _host_prep(x, wq, wk, wv, wo, rope_angles, anchor_indices)
    res = run_bass_kernel_spmd(nc, in_maps, core_ids=list(range(NCORES)),
                               **run_kwargs)
    acc = np.zeros((E, S), np.float64)
    for c in range(NCORES):
        acc += res.results[c]["outT"].astype(np.float64)
    out = np.ascontiguousarray(acc.T.reshape(B, S, E)).astype(np.float32)
    kernel.last_results = res
    return out


# revision 13
# speedup vs baseline: 2.1880x; 1.8506x over previous
"""KascadeReuseAttention Trainium2 kernel (v2).

Sharding: 16 heads / 8 cores -> 2 heads per core (head/tensor parallel).
Wq/Wk/Wv column-sharded by head, Wo row-sharded; host sums the 8 partial
outputs (the row-parallel all-reduce).

Single SPMD program for all cores: per-core anchor selection enters only as
DATA (per-tile multiplicity weight columns), never as program structure.
Per (head, query-tile t) we compute block attention against ALL past key
tiles v<=t and weight each tile's probabilities by m[h,t,v] = multiplicity
of v in {anchors} + {t} (0 if unselected) before the denominator and PV
matmuls. This reproduces the reference exactly (duplicate anchors included)
while keeping K/V resident in SBUF (no DRAM gather round trip).

v2 changes vs v1 (all aimed at the Tensor engine, the measured bottleneck):
- PV matmul flipped to produce out[q, d] (lhsT=wt, rhs=v_tile), so the
  softmax denominator is a 1-column matmul (rhs=ones) sharing wt as
  stationary, instead of a full 128-column stream per tile pair.
- Normalization uses a per-partition reciprocal [q,1] + tensor_scalar
  multiply; the f32 broadcast matmul is gone. A cheap PE transpose brings
  attn back to [d, s] layout for the output projection.
- Phase-1 q/k matmuls process chunk pairs with the same stationary weight
  tile back-to-back (lets the backend reuse LDWEIGHTS if it can).
- Output partials are written bf16 (halves the output DMA).
- Elementwise work is spread across DVE / Activation / GpSimd.
"""

import math
import sys

import numpy as np

for _p in ("/opt/trn_rl_repo",):
    if _p not in sys.path:
        sys.path.insert(0, _p)

import ml_dtypes  # noqa: E402
import concourse.bass as bass  # noqa: E402
import concourse.mybir as mybir  # noqa: E402
import concourse.tile as tile  # noqa: E402
from concourse.ap import AP  # noqa: E402
from concourse.bass_utils import run_bass_kernel_spmd  # noqa: E402
from concourse.vector_clock import ScopedClock  # noqa: E402

BF16 = mybir.dt.bfloat16
F32 = mybir.dt.float32
NPBF16 = ml_dtypes.bfloat16

B, S, E, H, D, K = 1, 4096, 2048, 16, 128, 8
TILE = 128
T = S // TILE          # 32 query/key tiles
NCORES = 8
HPC = H // NCORES      # heads per core = 2
CHUNK = 512            # s-chunk for projections
NCHUNK = S // CHUNK
SM_SCALE = 1.0 / math.sqrt(D)
NTRI = T * (T + 1) // 2  # 528 (t,v<=t) pairs per head

_PATCHED = False


def _patch_tile_drain():
    """This container's walrus caps per-instruction sync waits; the Tile
    kernel-tail drain carries one wait per live semaphore. Split them onto
    preceding SP nops."""
    global _PATCHED
    if _PATCHED:
        return
    _PATCHED = True

    def _drain_and_barrier(self, tick_clock, wait_clock):
        nc = self.nc
        nops = []
        nsems = len(self.sems.allocated()) if self.sems is not None else 0
        for _ in range(nsems):
            nops.append(nc.sync.nop())
        drain_inst = nc.sync.drain()
        wait_clock.add_sem_waits(
            drain_inst.ins, ScopedClock({None: tick_clock.global_clock})
        )
        si = drain_inst.ins.sync_info
        waits = list(si.on_wait or [])
        if len(waits) > 1:
            si.on_wait = waits[:1]
            for i, w in enumerate(waits[1:]):
                ni = nops[i].ins if hasattr(nops[i], "ins") else nops[i]
                nsi = ni.sync_info
                if nsi is None:
                    ni.sync_info = mybir.SyncInfo(on_wait=[w], on_update=[])
                else:
                    nsi.on_wait = [w]
        nc.all_engine_barrier()
        assert self.sems is not None
        popped = nc._tile_sem_poison_stack.pop()
        assert popped is self._sem_poison
        nc.clear_and_free_semaphores(list(self.sems.allocated().values()))
        nc.all_engine_barrier()
        _split_multi_waits(nc)

    tile.TileContext._drain_and_barrier = _drain_and_barrier


def _split_multi_waits(nc):
    """Walrus here encodes at most one sync-wait per instruction; move the
    extras onto preceding same-engine no-ops."""
    ctr = [0]
    for f in nc.m.functions:
        for bb in f.blocks:
            insts = list(bb.instructions)
            if not any(
                i.sync_info and i.sync_info.on_wait
                and len(i.sync_info.on_wait) > 1
                for i in insts
            ):
                continue
            newl = []
            for inst in insts:
                si = inst.sync_info
                if si and si.on_wait and len(si.on_wait) > 1:
                    waits = list(si.on_wait)
                    for w in waits[:-1]:
                        ctr[0] += 1
                        nop = mybir.InstNoOp(
                            name=f"WSPL-{ctr[0]}", ins=[], outs=[])
                        nop.engine = inst.engine
                        nop.sync_info = mybir.SyncInfo(
                            on_wait=[w], on_update=[])
                        newl.append(nop)
                    si.on_wait = waits[-1:]
                newl.append(inst)
            bb.instructions = newl


def _tri_col(t, v):
    return t * (t + 1) // 2 + v


def build_bass():
    """Uniform per-core program. Inputs (per core, bf16 unless noted):
    xT [E, S], wqk [E, 4*128] (q_h0,q_h1,k_h0,k_h1), wv [E, 256],
    wo [256, E], cosT/sinT [128, S], rotT [128,128] (R^T for rotate_half),
    triT [128,128] (tri[l,q] = l<=q), idT [128,128] identity,
    mw [128, HPC*NTRI] f32 weight columns.
    Output: outT [E, S] bf16 (partial x@.. contribution of this core's heads).
    """
    nc = bass.Bass()
    xT = nc.dram_tensor("xT", [E, S], BF16, kind="ExternalInput")
    wqk = nc.dram_tensor("wqk", [E, 4 * TILE], BF16, kind="ExternalInput")
    wv = nc.dram_tensor("wv", [E, 2 * TILE], BF16, kind="ExternalInput")
    wo = nc.dram_tensor("wo", [2 * TILE, E], BF16, kind="ExternalInput")
    cosT = nc.dram_tensor("cosT", [TILE, S], BF16, kind="ExternalInput")
    sinT = nc.dram_tensor("sinT", [TILE, S], BF16, kind="ExternalInput")
    rotT = nc.dram_tensor("rotT", [TILE, TILE], BF16, kind="ExternalInput")
    triT = nc.dram_tensor("triT", [TILE, TILE], BF16, kind="ExternalInput")
    idT = nc.dram_tensor("idT", [TILE, TILE], BF16, kind="ExternalInput")
    mw = nc.dram_tensor("mw", [TILE, HPC * NTRI], F32, kind="ExternalInput")
    outT = nc.dram_tensor("outT", [E, S], BF16, kind="ExternalOutput")

    EK = E // TILE  # 16 contraction tiles

    with tile.TileContext(nc) as tc:
        with tc.tile_pool(name="const", bufs=1) as cpool:
            sb_wqk = cpool.tile([TILE, EK, 4 * TILE], BF16)
            sb_wv = cpool.tile([TILE, EK, 2 * TILE], BF16)
            sb_wo = cpool.tile([TILE, 2, E], BF16)
            sb_cos = cpool.tile([TILE, S], BF16)
            sb_sin = cpool.tile([TILE, S], BF16)
            sb_rot = cpool.tile([TILE, TILE], BF16)
            sb_tri = cpool.tile([TILE, TILE], BF16)
            sb_id = cpool.tile([TILE, TILE], BF16)
            sb_mw = cpool.tile([TILE, HPC * NTRI], F32)
            ones_col = cpool.tile([TILE, 1], BF16)
            # persistent per-head tensors (bf16): qT/kT [d, S], v [s-tiles, d]
            sb_q = cpool.tile([TILE, HPC, S], BF16, tag="q")
            sb_k = cpool.tile([TILE, HPC, S], BF16, tag="k")
            sb_v = cpool.tile([TILE, HPC, S], BF16, tag="v")
            sb_attn = cpool.tile([TILE, HPC, S], BF16, tag="attn")

            nc.sync.dma_start(out=sb_wqk[:],
                              in_=wqk.rearrange("(a p) b -> p a b", p=TILE))
            nc.sync.dma_start(out=sb_wv[:],
                              in_=wv.rearrange("(a p) b -> p a b", p=TILE))
            nc.sync.dma_start(out=sb_wo[:],
                              in_=wo.rearrange("(a p) b -> p a b", p=TILE))
            nc.sync.dma_start(out=sb_cos[:], in_=cosT[:])
            nc.sync.dma_start(out=sb_sin[:], in_=sinT[:])
            nc.sync.dma_start(out=sb_rot[:], in_=rotT[:])
            nc.sync.dma_start(out=sb_tri[:], in_=triT[:])
            nc.sync.dma_start(out=sb_id[:], in_=idT[:])
            nc.sync.dma_start(out=sb_mw[:], in_=mw[:])
            nc.vector.memset(ones_col[:], 1.0)

            # ---------------- Phase 1: projections + RoPE ----------------
            # Chunk pairs: for each m-tile the two chunks' accumulation
            # steps share the stationary weight tile back-to-back.
            with (
                tc.tile_pool(name="xin", bufs=2) as xpool,
                tc.tile_pool(name="ptmp", bufs=3) as tpool,
                tc.tile_pool(name="qkps", bufs=2, space="PSUM") as qkps,
                tc.tile_pool(name="vps", bufs=2, space="PSUM") as vps,
                tc.tile_pool(name="rops", bufs=2, space="PSUM") as rops,
            ):
                for cp in range(NCHUNK // 2):
                    xts = []
                    for half in range(2):
                        s0 = (2 * cp + half) * CHUNK
                        xt = xpool.tile([TILE, EK, CHUNK], BF16, tag="xt")
                        nc.sync.dma_start(
                            out=xt[:],
                            in_=xT[:, s0:s0 + CHUNK].rearrange(
                                "(a p) b -> p a b", p=TILE),
                        )
                        xts.append(xt)
                    # qT/kT M-tiles: 0=q_h0 1=q_h1 2=k_h0 3=k_h1
                    for m in range(4):
                        pss = [qkps.tile([TILE, CHUNK], F32, tag="qk",
                                         name=f"qk{m}a"),
                               qkps.tile([TILE, CHUNK], F32, tag="qk",
                                         name=f"qk{m}b")]
                        for e in range(EK):
                            for half in range(2):
                                nc.tensor.matmul(
                                    pss[half][:],
                                    sb_wqk[:, e, m * TILE:(m + 1) * TILE],
                                    xts[half][:, e, :], start=(e == 0),
                                    stop=(e == EK - 1),
                                    skip_group_check=True)
                        for half in range(2):
                            s0 = (2 * cp + half) * CHUNK
                            ps = pss[half]
                            raw = tpool.tile([TILE, CHUNK], BF16, tag="raw")
                            nc.scalar.copy(out=raw[:], in_=ps[:])
                            rot = rops.tile([TILE, CHUNK], F32, tag="rot")
                            nc.tensor.matmul(rot[:], sb_rot[:], raw[:],
                                             start=True, stop=True)
                            t1 = tpool.tile([TILE, CHUNK], BF16, tag="t1")
                            nc.gpsimd.tensor_mul(t1[:], raw[:],
                                                 sb_cos[:, s0:s0 + CHUNK])
                            t2 = tpool.tile([TILE, CHUNK], BF16, tag="t2")
                            nc.vector.tensor_mul(t2[:], rot[:],
                                                 sb_sin[:, s0:s0 + CHUNK])
                            dst = sb_q if m < 2 else sb_k
                            h = m % 2
                            nc.vector.tensor_add(dst[:, h, s0:s0 + CHUNK],
                                                 t1[:], t2[:])
                    # v: M-tiles over s (8 per chunk pair), N = 2 heads * 128
                    for sm in range(2 * CHUNK // TILE):
                        vp = vps.tile([TILE, 2 * TILE], F32, tag="v")
                        st = 2 * cp * CHUNK + sm * TILE
                        xt = xts[sm // (CHUNK // TILE)]
                        lo = (sm % (CHUNK // TILE)) * TILE
                        for e in range(EK):
                            nc.tensor.matmul(
                                vp[:], xt[:, e, lo:lo + TILE],
                                sb_wv[:, e, :], start=(e == 0),
                                stop=(e == EK - 1))
                        for h in range(HPC):
                            if h == 0:
                                nc.scalar.copy(
                                    out=sb_v[:, h, st:st + TILE],
                                    in_=vp[:, h * TILE:(h + 1) * TILE])
                            else:
                                nc.vector.tensor_copy(
                                    sb_v[:, h, st:st + TILE],
                                    vp[:, h * TILE:(h + 1) * TILE])

            # ---------------- Phase 2: block-sparse attention ------------
            # Per (h, t): logits [l,q] per past tile v, exp, multiplicity
            # weight, then PV flipped to out[q, d] with wt stationary so the
            # denominator is a 1-column matmul. Normalize with a [q,1]
            # reciprocal, transpose back to [d, q] on the PE.
            GRP = 4  # logits tiles per psum bank
            with (
                tc.tile_pool(name="wt", bufs=3) as wtp,
                tc.tile_pool(name="nrm", bufs=3) as nrm,
                tc.tile_pool(name="lg", bufs=2, space="PSUM") as lgps,
                tc.tile_pool(name="ot", bufs=2, space="PSUM") as otps,
                tc.tile_pool(name="dn", bufs=2, space="PSUM") as dnps,
                tc.tile_pool(name="tp", bufs=2, space="PSUM") as tpps,
            ):
                pending = None  # (at_tile, h, t) awaiting transpose
                for h in range(HPC):
                    for t in range(T):
                        nv = t + 1
                        q_sl = sb_q[:, h, t * TILE:(t + 1) * TILE]
                        out_ps = otps.tile([TILE, TILE], F32, tag="ot")
                        den_ps = dnps.tile([TILE, 1], F32, tag="dn")
                        ngrp = (nv + GRP - 1) // GRP
                        first = True
                        for g in range(ngrp):
                            v0 = g * GRP
                            gn = min(GRP, nv - v0)
                            lg = lgps.tile([TILE, GRP * TILE], F32, tag="lg")
                            for j in range(gn):
                                v = v0 + j
                                nc.tensor.matmul(
                                    lg[:, j * TILE:(j + 1) * TILE],
                                    sb_k[:, h, v * TILE:(v + 1) * TILE],
                                    q_sl, start=True, stop=True)
                            wt = wtp.tile([TILE, GRP * TILE], BF16, tag="wt")
                            nc.scalar.activation(
                                out=wt[:, :gn * TILE], in_=lg[:, :gn * TILE],
                                func=mybir.ActivationFunctionType.Exp,
                                scale=SM_SCALE)
                            # multiplicity weights for the whole group in one
                            # broadcast multiply (mw columns for (t, v0..)
                            # are contiguous); diagonal tile gets the causal
                            # tri-mask separately after.
                            wta = wt[:, :gn * TILE].rearrange(
                                "p (s q) -> p s q", q=TILE)
                            c0 = h * NTRI + _tri_col(t, v0)
                            mb = sb_mw[:, c0:c0 + gn]
                            mbc = AP(tensor=mb.tensor, offset=mb.offset,
                                     ap=[list(p) for p in mb.ap]
                                     + [[0, TILE]])
                            nc.vector.tensor_mul(wta, wta, mbc)
                            if v0 + gn - 1 == t:
                                wsl = wt[:, (gn - 1) * TILE:gn * TILE]
                                nc.vector.tensor_mul(wsl, wsl, sb_tri[:])
                            for j in range(gn):
                                v = v0 + j
                                last = (g == ngrp - 1) and (j == gn - 1)
                                wsl = wt[:, j * TILE:(j + 1) * TILE]
                                # out[q, d] += wt^T-contracted v tile;
                                # den[q, 1] shares wt as stationary.
                                nc.tensor.matmul(
                                    out_ps[:], wsl,
                                    sb_v[:, h, v * TILE:(v + 1) * TILE],
                                    start=first, stop=last,
                                    skip_group_check=True)
                                nc.tensor.matmul(
                                    den_ps[:], wsl, ones_col[:],
                                    start=first, stop=last,
                                    skip_group_check=True)
                                first = False
                        # normalize: per-partition recip (DVE), scale on ACT
                        rc = nrm.tile([TILE, 1], F32, tag="rc")
                        nc.vector.reciprocal(out=rc[:], in_=den_ps[:])
                        at = nrm.tile([TILE, TILE], BF16, tag="at")
                        nc.scalar.activation(
                            out=at[:], in_=out_ps[:],
                            func=mybir.ActivationFunctionType.Copy,
                            scale=rc[:])
                        # transpose back to [d, q] one iteration late so the
                        # PE never stalls on the recip/normalize chain
                        if pending is not None:
                            pat, ph, pt = pending
                            tp = tpps.tile([TILE, TILE], BF16, tag="tp")
                            nc.tensor.transpose(tp[:], pat[:], sb_id[:])
                            nc.scalar.copy(
                                out=sb_attn[:, ph, pt * TILE:(pt + 1) * TILE],
                                in_=tp[:])
                        pending = (at, h, t)
                if pending is not None:
                    pat, ph, pt = pending
                    tp = tpps.tile([TILE, TILE], BF16, tag="tp")
                    nc.tensor.transpose(tp[:], pat[:], sb_id[:])
                    nc.scalar.copy(
                        out=sb_attn[:, ph, pt * TILE:(pt + 1) * TILE],
                        in_=tp[:])

            # ---------------- Phase 3: output projection -----------------
            # One big DMA per chunk (the per-m-tile DMAs cost ~700ns of SP
            # issue each and left a long idle drain tail).
            with (
                tc.tile_pool(name="ost", bufs=2) as ost,
                tc.tile_pool(name="wops", bufs=3, space="PSUM") as wops,
            ):
                for ci in range(NCHUNK):
                    s0 = ci * CHUNK
                    ob = ost.tile([TILE, EK, CHUNK], BF16, tag="ob")
                    for m in range(EK):  # output e tiles
                        op = wops.tile([TILE, CHUNK], F32, tag="op")
                        for h in range(HPC):
                            nc.tensor.matmul(
                                op[:], sb_wo[:, h, m * TILE:(m + 1) * TILE],
                                sb_attn[:, h, s0:s0 + CHUNK],
                                start=(h == 0), stop=(h == HPC - 1))
                        if m % 2 == 0:
                            nc.scalar.copy(out=ob[:, m, :], in_=op[:])
                        else:
                            nc.vector.tensor_copy(ob[:, m, :], op[:])
                    nc.sync.dma_start(
                        out=outT[:, s0:s0 + CHUNK].rearrange(
                            "(a p) b -> p a b", p=TILE),
                        in_=ob[:])
    return nc


def _host_prep(x, wq, wk, wv, wo, rope_angles, anchor_indices):
    xT = np.ascontiguousarray(x[0].T).astype(NPBF16)
    cos = np.cos(rope_angles.astype(np.float64))
    sin = np.sin(rope_angles.astype(np.float64))
    cosT = np.ascontiguousarray(
        np.concatenate([cos, cos], axis=1).T).astype(NPBF16)
    sinT = np.ascontiguousarray(
        np.concatenate([sin, sin], axis=1).T).astype(NPBF16)
    half = D // 2
    R = np.zeros((D, D), np.float32)
    for d in range(half):
        R[d, d + half] = -1.0
        R[d + half, d] = 1.0
    rotT = np.ascontiguousarray(R.T).astype(NPBF16)
    tri = (np.arange(TILE)[:, None] <= np.arange(TILE)[None, :])
    triT = tri.astype(NPBF16)
    idT = np.eye(TILE, dtype=NPBF16)

    in_maps = []
    for c in range(NCORES):
        heads = [c * HPC + i for i in range(HPC)]
        wqk_c = np.concatenate(
            [wq[:, h * D:(h + 1) * D] for h in heads]
            + [wk[:, h * D:(h + 1) * D] for h in heads], axis=1)
        wv_c = np.concatenate([wv[:, h * D:(h + 1) * D] for h in heads],
                              axis=1)
        wo_c = np.concatenate([wo[h * D:(h + 1) * D, :] for h in heads],
                              axis=0)
        mwc = np.zeros((TILE, HPC * NTRI), np.float32)
        for i, h in enumerate(heads):
            for t in range(T):
                sel = list(anchor_indices[0, h, t]) + [t]
                for v in range(t + 1):
                    m = sel.count(v)
                    if m:
                        mwc[:, i * NTRI + _tri_col(t, v)] = float(m)
        in_maps.append({
            "xT": xT, "wqk": np.ascontiguousarray(wqk_c).astype(NPBF16),
            "wv": np.ascontiguousarray(wv_c).astype(NPBF16),
            "wo": np.ascontiguousarray(wo_c).astype(NPBF16),
            "cosT": cosT, "sinT": sinT, "rotT": rotT, "triT": triT,
            "idT": idT, "mw": mwc,
        })
    return in_maps


def kernel(x, wq, wk, wv, wo, rope_angles, anchor_indices, **run_kwargs):
    _patch_tile_drain()
    nc = build_bass()
    in_maps = _host_prep(x, wq, wk, wv, wo, rope_angles, anchor_indices)
    res = run_bass_kernel_spmd(nc, in_maps, core_ids=list(range(NCORES)),
                               **run_kwargs)
    acc = np.zeros((E, S), np.float64)
    for c in range(NCORES):
        acc += res.results[c]["outT"].astype(np.float64)
    out = np.ascontiguousarray(acc.T.reshape(B, S, E)).astype(np.float32)
    kernel.last_results = res
    return out


# revision 15
# speedup vs baseline: 2.3061x; 1.0540x over previous
"""KascadeReuseAttention Trainium2 kernel (v2).

Sharding: 16 heads / 8 cores -> 2 heads per core (head/tensor parallel).
Wq/Wk/Wv column-sharded by head, Wo row-sharded; host sums the 8 partial
outputs (the row-parallel all-reduce).

Single SPMD program for all cores: per-core anchor selection enters only as
DATA (per-tile multiplicity weight columns), never as program structure.
Per (head, query-tile t) we compute block attention against ALL past key
tiles v<=t and weight each tile's probabilities by m[h,t,v] = multiplicity
of v in {anchors} + {t} (0 if unselected) before the denominator and PV
matmuls. This reproduces the reference exactly (duplicate anchors included)
while keeping K/V resident in SBUF (no DRAM gather round trip).

v2 changes vs v1 (all aimed at the Tensor engine, the measured bottleneck):
- PV matmul flipped to produce out[q, d] (lhsT=wt, rhs=v_tile), so the
  softmax denominator is a 1-column matmul (rhs=ones) sharing wt as
  stationary, instead of a full 128-column stream per tile pair.
- Normalization uses a per-partition reciprocal [q,1] + tensor_scalar
  multiply; the f32 broadcast matmul is gone. A cheap PE transpose brings
  attn back to [d, s] layout for the output projection.
- Phase-1 q/k matmuls process chunk pairs with the same stationary weight
  tile back-to-back (lets the backend reuse LDWEIGHTS if it can).
- Output partials are written bf16 (halves the output DMA).
- Elementwise work is spread across DVE / Activation / GpSimd.
"""

import math
import sys

import numpy as np

for _p in ("/opt/trn_rl_repo",):
    if _p not in sys.path:
        sys.path.insert(0, _p)

import ml_dtypes  # noqa: E402
import concourse.bass as bass  # noqa: E402
import concourse.mybir as mybir  # noqa: E402
import concourse.tile as tile  # noqa: E402
from concourse.bass_utils import run_bass_kernel_spmd  # noqa: E402
from concourse.vector_clock import ScopedClock  # noqa: E402

BF16 = mybir.dt.bfloat16
F32 = mybir.dt.float32
NPBF16 = ml_dtypes.bfloat16

B, S, E, H, D, K = 1, 4096, 2048, 16, 128, 8
TILE = 128
T = S // TILE          # 32 query/key tiles
NCORES = 8
HPC = H // NCORES      # heads per core = 2
CHUNK = 512            # s-chunk for projections
NCHUNK = S // CHUNK
SM_SCALE = 1.0 / math.sqrt(D)
NTRI = T * (T + 1) // 2  # 528 (t,v<=t) pairs per head

_PATCHED = False


def _patch_tile_drain():
    """This container's walrus caps per-instruction sync waits; the Tile
    kernel-tail drain carries one wait per live semaphore. Split them onto
    preceding SP nops."""
    global _PATCHED
    if _PATCHED:
        return
    _PATCHED = True

    def _drain_and_barrier(self, tick_clock, wait_clock):
        nc = self.nc
        nops = []
        nsems = len(self.sems.allocated()) if self.sems is not None else 0
        for _ in range(nsems):
            nops.append(nc.sync.nop())
        drain_inst = nc.sync.drain()
        wait_clock.add_sem_waits(
            drain_inst.ins, ScopedClock({None: tick_clock.global_clock})
        )
        si = drain_inst.ins.sync_info
        waits = list(si.on_wait or [])
        if len(waits) > 1:
            si.on_wait = waits[:1]
            for i, w in enumerate(waits[1:]):
                ni = nops[i].ins if hasattr(nops[i], "ins") else nops[i]
                nsi = ni.sync_info
                if nsi is None:
                    ni.sync_info = mybir.SyncInfo(on_wait=[w], on_update=[])
                else:
                    nsi.on_wait = [w]
        nc.all_engine_barrier()
        assert self.sems is not None
        popped = nc._tile_sem_poison_stack.pop()
        assert popped is self._sem_poison
        nc.clear_and_free_semaphores(list(self.sems.allocated().values()))
        nc.all_engine_barrier()
        _split_multi_waits(nc)

    tile.TileContext._drain_and_barrier = _drain_and_barrier


def _split_multi_waits(nc):
    """Walrus here encodes at most one sync-wait per instruction; move the
    extras onto preceding same-engine no-ops."""
    ctr = [0]
    for f in nc.m.functions:
        for bb in f.blocks:
            insts = list(bb.instructions)
            if not any(
                i.sync_info and i.sync_info.on_wait
                and len(i.sync_info.on_wait) > 1
                for i in insts
            ):
                continue
            newl = []
            for inst in insts:
                si = inst.sync_info
                if si and si.on_wait and len(si.on_wait) > 1:
                    waits = list(si.on_wait)
                    for w in waits[:-1]:
                        ctr[0] += 1
                        nop = mybir.InstNoOp(
                            name=f"WSPL-{ctr[0]}", ins=[], outs=[])
                        nop.engine = inst.engine
                        nop.sync_info = mybir.SyncInfo(
                            on_wait=[w], on_update=[])
                        newl.append(nop)
                    si.on_wait = waits[-1:]
                newl.append(inst)
            bb.instructions = newl


def _tri_col(t, v):
    return t * (t + 1) // 2 + v


def build_bass():
    """Uniform per-core program. Inputs (per core, bf16 unless noted):
    xT [E, S], wqk [E, 4*128] (q_h0,q_h1,k_h0,k_h1), wv [E, 256],
    wo [256, E], cosT/sinT [128, S], rotT [128,128] (R^T for rotate_half),
    triT [128,128] (tri[l,q] = l<=q), idT [128,128] identity,
    mw [128, HPC*NTRI] f32 weight columns.
    Output: outT [E, S] bf16 (partial x@.. contribution of this core's heads).
    """
    nc = bass.Bass()
    xT = nc.dram_tensor("xT", [E, S], BF16, kind="ExternalInput")
    wqk = nc.dram_tensor("wqk", [E, 4 * TILE], BF16, kind="ExternalInput")
    wv = nc.dram_tensor("wv", [E, 2 * TILE], BF16, kind="ExternalInput")
    wo = nc.dram_tensor("wo", [2 * TILE, E], BF16, kind="ExternalInput")
    cosT = nc.dram_tensor("cosT", [TILE, S], BF16, kind="ExternalInput")
    sinT = nc.dram_tensor("sinT", [TILE, S], BF16, kind="ExternalInput")
    rotT = nc.dram_tensor("rotT", [TILE, TILE], BF16, kind="ExternalInput")
    triT = nc.dram_tensor("triT", [TILE, TILE], BF16, kind="ExternalInput")
    idT = nc.dram_tensor("idT", [TILE, TILE], BF16, kind="ExternalInput")
    mw = nc.dram_tensor("mw", [TILE, HPC * NTRI], F32, kind="ExternalInput")
    outT = nc.dram_tensor("outT", [E, S], BF16, kind="ExternalOutput")

    EK = E // TILE  # 16 contraction tiles

    with tile.TileContext(nc) as tc:
        with tc.tile_pool(name="const", bufs=1) as cpool:
            sb_wqk = cpool.tile([TILE, EK, 4 * TILE], BF16)
            sb_wv = cpool.tile([TILE, EK, 2 * TILE], BF16)
            sb_wo = cpool.tile([TILE, 2, E], BF16)
            sb_cos = cpool.tile([TILE, S], BF16)
            sb_sin = cpool.tile([TILE, S], BF16)
            sb_rot = cpool.tile([TILE, TILE], BF16)
            sb_tri = cpool.tile([TILE, TILE], BF16)
            sb_id = cpool.tile([TILE, TILE], BF16)
            sb_mw = cpool.tile([TILE, HPC * NTRI], F32)
            ones_col = cpool.tile([TILE, 1], BF16)
            # persistent per-head tensors (bf16): qT/kT [d, S], v [s-tiles, d]
            sb_q = cpool.tile([TILE, HPC, S], BF16, tag="q")
            sb_k = cpool.tile([TILE, HPC, S], BF16, tag="k")
            sb_v = cpool.tile([TILE, HPC, S], BF16, tag="v")
            sb_attn = cpool.tile([TILE, HPC, S], BF16, tag="attn")

            nc.sync.dma_start(out=sb_wqk[:],
                              in_=wqk.rearrange("(a p) b -> p a b", p=TILE))
            nc.sync.dma_start(out=sb_wv[:],
                              in_=wv.rearrange("(a p) b -> p a b", p=TILE))
            nc.sync.dma_start(out=sb_wo[:],
                              in_=wo.rearrange("(a p) b -> p a b", p=TILE))
            nc.sync.dma_start(out=sb_cos[:], in_=cosT[:])
            nc.sync.dma_start(out=sb_sin[:], in_=sinT[:])
            nc.sync.dma_start(out=sb_rot[:], in_=rotT[:])
            nc.sync.dma_start(out=sb_tri[:], in_=triT[:])
            nc.sync.dma_start(out=sb_id[:], in_=idT[:])
            nc.sync.dma_start(out=sb_mw[:], in_=mw[:])
            nc.vector.memset(ones_col[:], 1.0)

            # ---------------- Phase 1: projections + RoPE ----------------
            # Chunk pairs: for each m-tile the two chunks' accumulation
            # steps share the stationary weight tile back-to-back.
            with (
                tc.tile_pool(name="xin", bufs=2) as xpool,
                tc.tile_pool(name="ptmp", bufs=3) as tpool,
                tc.tile_pool(name="qkps", bufs=2, space="PSUM") as qkps,
                tc.tile_pool(name="vps", bufs=2, space="PSUM") as vps,
                tc.tile_pool(name="rops", bufs=2, space="PSUM") as rops,
            ):
                for cp in range(NCHUNK // 2):
                    xts = []
                    for half in range(2):
                        s0 = (2 * cp + half) * CHUNK
                        xt = xpool.tile([TILE, EK, CHUNK], BF16, tag="xt")
                        nc.sync.dma_start(
                            out=xt[:],
                            in_=xT[:, s0:s0 + CHUNK].rearrange(
                                "(a p) b -> p a b", p=TILE),
                        )
                        xts.append(xt)
                    # qT/kT M-tiles: 0=q_h0 1=q_h1 2=k_h0 3=k_h1
                    for m in range(4):
                        pss = [qkps.tile([TILE, CHUNK], F32, tag="qk",
                                         name=f"qk{m}a"),
                               qkps.tile([TILE, CHUNK], F32, tag="qk",
                                         name=f"qk{m}b")]
                        for e in range(EK):
                            for half in range(2):
                                nc.tensor.matmul(
                                    pss[half][:],
                                    sb_wqk[:, e, m * TILE:(m + 1) * TILE],
                                    xts[half][:, e, :], start=(e == 0),
                                    stop=(e == EK - 1),
                                    skip_group_check=True)
                        for half in range(2):
                            s0 = (2 * cp + half) * CHUNK
                            ps = pss[half]
                            raw = tpool.tile([TILE, CHUNK], BF16, tag="raw")
                            nc.scalar.copy(out=raw[:], in_=ps[:])
                            rot = rops.tile([TILE, CHUNK], F32, tag="rot")
                            nc.tensor.matmul(rot[:], sb_rot[:], raw[:],
                                             start=True, stop=True)
                            t1 = tpool.tile([TILE, CHUNK], BF16, tag="t1")
                            nc.gpsimd.tensor_mul(t1[:], raw[:],
                                                 sb_cos[:, s0:s0 + CHUNK])
                            t2 = tpool.tile([TILE, CHUNK], BF16, tag="t2")
                            nc.vector.tensor_mul(t2[:], rot[:],
                                                 sb_sin[:, s0:s0 + CHUNK])
                            dst = sb_q if m < 2 else sb_k
                            h = m % 2
                            nc.vector.tensor_add(dst[:, h, s0:s0 + CHUNK],
                                                 t1[:], t2[:])
                    # v: M-tiles over s (8 per chunk pair), N = 2 heads * 128
                    for sm in range(2 * CHUNK // TILE):
                        vp = vps.tile([TILE, 2 * TILE], F32, tag="v")
                        st = 2 * cp * CHUNK + sm * TILE
                        xt = xts[sm // (CHUNK // TILE)]
                        lo = (sm % (CHUNK // TILE)) * TILE
                        for e in range(EK):
                            nc.tensor.matmul(
                                vp[:], xt[:, e, lo:lo + TILE],
                                sb_wv[:, e, :], start=(e == 0),
                                stop=(e == EK - 1))
                        for h in range(HPC):
                            if h == 0:
                                nc.scalar.copy(
                                    out=sb_v[:, h, st:st + TILE],
                                    in_=vp[:, h * TILE:(h + 1) * TILE])
                            else:
                                nc.vector.tensor_copy(
                                    sb_v[:, h, st:st + TILE],
                                    vp[:, h * TILE:(h + 1) * TILE])

            # ---------------- Phase 2: block-sparse attention ------------
            # Per (h, t): logits [l,q] per past tile v, exp, multiplicity
            # weight, then PV flipped to out[q, d] with wt stationary so the
            # denominator is a 1-column matmul. Normalize with a [q,1]
            # reciprocal, transpose back to [d, q] on the PE.
            GRP = 4  # logits tiles per psum bank
            with (
                tc.tile_pool(name="wt", bufs=3) as wtp,
                tc.tile_pool(name="nrm", bufs=3) as nrm,
                tc.tile_pool(name="lg", bufs=2, space="PSUM") as lgps,
                tc.tile_pool(name="ot", bufs=2, space="PSUM") as otps,
                tc.tile_pool(name="dn", bufs=2, space="PSUM") as dnps,
                tc.tile_pool(name="tp", bufs=2, space="PSUM") as tpps,
            ):
                pending = None  # (at_tile, h, t) awaiting transpose
                for h in range(HPC):
                    for t in range(T):
                        nv = t + 1
                        q_sl = sb_q[:, h, t * TILE:(t + 1) * TILE]
                        out_ps = otps.tile([TILE, TILE], F32, tag="ot")
                        den_ps = dnps.tile([TILE, 1], F32, tag="dn")
                        ngrp = (nv + GRP - 1) // GRP
                        first = True
                        for g in range(ngrp):
                            v0 = g * GRP
                            gn = min(GRP, nv - v0)
                            lg = lgps.tile([TILE, GRP * TILE], F32, tag="lg")
                            for j in range(gn):
                                v = v0 + j
                                nc.tensor.matmul(
                                    lg[:, j * TILE:(j + 1) * TILE],
                                    sb_k[:, h, v * TILE:(v + 1) * TILE],
                                    q_sl, start=True, stop=True)
                            wt = wtp.tile([TILE, GRP * TILE], BF16, tag="wt")
                            nc.scalar.activation(
                                out=wt[:, :gn * TILE], in_=lg[:, :gn * TILE],
                                func=mybir.ActivationFunctionType.Exp,
                                scale=SM_SCALE)
                            for j in range(gn):
                                v = v0 + j
                                mcol = sb_mw[:, h * NTRI + _tri_col(t, v):
                                             h * NTRI + _tri_col(t, v) + 1]
                                wsl = wt[:, j * TILE:(j + 1) * TILE]
                                if v == t:
                                    nc.vector.scalar_tensor_tensor(
                                        out=wsl, in0=wsl, scalar=mcol,
                                        in1=sb_tri[:],
                                        op0=mybir.AluOpType.mult,
                                        op1=mybir.AluOpType.mult)
                                else:
                                    nc.vector.tensor_scalar_mul(wsl, wsl,
                                                                mcol)
                            for j in range(gn):
                                v = v0 + j
                                last = (g == ngrp - 1) and (j == gn - 1)
                                wsl = wt[:, j * TILE:(j + 1) * TILE]
                                # out[q, d] += wt^T-contracted v tile;
                                # den[q, 1] shares wt as stationary.
                                nc.tensor.matmul(
                                    out_ps[:], wsl,
                                    sb_v[:, h, v * TILE:(v + 1) * TILE],
                                    start=first, stop=last,
                                    skip_group_check=True)
                                nc.tensor.matmul(
                                    den_ps[:], wsl, ones_col[:],
                                    start=first, stop=last,
                                    skip_group_check=True)
                                first = False
                        # normalize: per-partition recip (DVE), scale on ACT
                        rc = nrm.tile([TILE, 1], F32, tag="rc")
                        nc.vector.reciprocal(out=rc[:], in_=den_ps[:])
                        at = nrm.tile([TILE, TILE], BF16, tag="at")
                        nc.scalar.activation(
                            out=at[:], in_=out_ps[:],
                            func=mybir.ActivationFunctionType.Copy,
                            scale=rc[:])
                        # transpose back to [d, q] one iteration late so the
                        # PE never stalls on the recip/normalize chain
                        if pending is not None:
                            pat, ph, pt = pending
                            tp = tpps.tile([TILE, TILE], BF16, tag="tp")
                            nc.tensor.transpose(tp[:], pat[:], sb_id[:])
                            nc.scalar.copy(
                                out=sb_attn[:, ph, pt * TILE:(pt + 1) * TILE],
                                in_=tp[:])
                        pending = (at, h, t)
                if pending is not None:
                    pat, ph, pt = pending
                    tp = tpps.tile([TILE, TILE], BF16, tag="tp")
                    nc.tensor.transpose(tp[:], pat[:], sb_id[:])
                    nc.scalar.copy(
                        out=sb_attn[:, ph, pt * TILE:(pt + 1) * TILE],
                        in_=tp[:])

            # ---------------- Phase 3: output projection -----------------
            with (
                tc.tile_pool(name="ost", bufs=3) as ost,
                tc.tile_pool(name="wops", bufs=3, space="PSUM") as wops,
            ):
                for ci in range(NCHUNK):
                    s0 = ci * CHUNK
                    for m in range(EK):  # output e tiles
                        op = wops.tile([TILE, CHUNK], F32, tag="op")
                        for h in range(HPC):
                            nc.tensor.matmul(
                                op[:], sb_wo[:, h, m * TILE:(m + 1) * TILE],
                                sb_attn[:, h, s0:s0 + CHUNK],
                                start=(h == 0), stop=(h == HPC - 1))
                        ob = ost.tile([TILE, CHUNK], BF16, tag="ob")
                        if m % 2 == 0:
                            nc.scalar.copy(out=ob[:], in_=op[:])
                            # ACT copied this tile; SP issues its DMA
                            nc.sync.dma_start(
                                out=outT[m * TILE:(m + 1) * TILE,
                                         s0:s0 + CHUNK],
                                in_=ob[:])
                        else:
                            nc.vector.tensor_copy(ob[:], op[:])
                            # split the output-DMA issue load: the ACT
                            # engine also fronts a HWDGE queue and idles
                            # during the phase-3 drain
                            nc.scalar.dma_start(
                                out=outT[m * TILE:(m + 1) * TILE,
                                         s0:s0 + CHUNK],
                                in_=ob[:])
    return nc


def _host_prep(x, wq, wk, wv, wo, rope_angles, anchor_indices):
    xT = np.ascontiguousarray(x[0].T).astype(NPBF16)
    cos = np.cos(rope_angles.astype(np.float64))
    sin = np.sin(rope_angles.astype(np.float64))
    cosT = np.ascontiguousarray(
        np.concatenate([cos, cos], axis=1).T).astype(NPBF16)
    sinT = np.ascontiguousarray(
        np.concatenate([sin, sin], axis=1).T).astype(NPBF16)
    half = D // 2
    R = np.zeros((D, D), np.float32)
    for d in range(half):
        R[d, d + half] = -1.0
        R[d + half, d] = 1.0
    rotT = np.ascontiguousarray(R.T).astype(NPBF16)
    tri = (np.arange(TILE)[:, None] <= np.arange(TILE)[None, :])
    triT = tri.astype(NPBF16)
    idT = np.eye(TILE, dtype=NPBF16)

    in_maps = []
    for c in range(NCORES):
        heads = [c * HPC + i for i in range(HPC)]
        wqk_c = np.concatenate(
            [wq[:, h * D:(h + 1) * D] for h in heads]
            + [wk[:, h * D:(h + 1) * D] for h in heads], axis=1)
        wv_c = np.concatenate([wv[:, h * D:(h + 1) * D] for h in heads],
                              axis=1)
        wo_c = np.concatenate([wo[h * D:(h + 1) * D, :] for h in heads],
                              axis=0)
        mwc = np.zeros((TILE, HPC * NTRI), np.float32)
        for i, h in enumerate(heads):
            for t in range(T):
                sel = list(anchor_indices[0, h, t]) + [t]
                for v in range(t + 1):
                    m = sel.count(v)
                    if m:
                        mwc[:, i * NTRI + _tri_col(t, v)] = float(m)
        in_maps.append({
            "xT": xT, "wqk": np.ascontiguousarray(wqk_c).astype(NPBF16),
            "wv": np.ascontiguousarray(wv_c).astype(NPBF16),
            "wo": np.ascontiguousarray(wo_c).astype(NPBF16),
            "cosT": cosT, "sinT": sinT, "rotT": rotT, "triT": triT,
            "idT": idT, "mw": mwc,
        })
    return in_maps


def kernel(x, wq, wk, wv, wo, rope_angles, anchor_indices, **run_kwargs):
    _patch_tile_drain()
    nc = build_bass()
    in_maps = _host_prep(x, wq, wk, wv, wo, rope_angles, anchor_indices)
    res = run_bass_kernel_spmd(nc, in_maps, core_ids=list(range(NCORES)),
                               **run_kwargs)
    acc = np.zeros((E, S), np.float64)
    for c in range(NCORES):
        acc += res.results[c]["outT"].astype(np.float64)
    out = np.ascontiguousarray(acc.T.reshape(B, S, E)).astype(np.float32)
    kernel.last_results = res
    return out


# revision 17
# speedup vs baseline: 2.3134x; 1.0032x over previous
"""KascadeReuseAttention Trainium2 kernel (v2).

Sharding: 16 heads / 8 cores -> 2 heads per core (head/tensor parallel).
Wq/Wk/Wv column-sharded by head, Wo row-sharded; host sums the 8 partial
outputs (the row-parallel all-reduce).

Single SPMD program for all cores: per-core anchor selection enters only as
DATA (per-tile multiplicity weight columns), never as program structure.
Per (head, query-tile t) we compute block attention against ALL past key
tiles v<=t and weight each tile's probabilities by m[h,t,v] = multiplicity
of v in {anchors} + {t} (0 if unselected) before the denominator and PV
matmuls. This reproduces the reference exactly (duplicate anchors included)
while keeping K/V resident in SBUF (no DRAM gather round trip).

v2 changes vs v1 (all aimed at the Tensor engine, the measured bottleneck):
- PV matmul flipped to produce out[q, d] (lhsT=wt, rhs=v_tile), so the
  softmax denominator is a 1-column matmul (rhs=ones) sharing wt as
  stationary, instead of a full 128-column stream per tile pair.
- Normalization uses a per-partition reciprocal [q,1] + tensor_scalar
  multiply; the f32 broadcast matmul is gone. A cheap PE transpose brings
  attn back to [d, s] layout for the output projection.
- Phase-1 q/k matmuls process chunk pairs with the same stationary weight
  tile back-to-back (lets the backend reuse LDWEIGHTS if it can).
- Output partials are written bf16 (halves the output DMA).
- Elementwise work is spread across DVE / Activation / GpSimd.
"""

import math
import sys

import numpy as np

for _p in ("/opt/trn_rl_repo",):
    if _p not in sys.path:
        sys.path.insert(0, _p)

import ml_dtypes  # noqa: E402
import concourse.bass as bass  # noqa: E402
import concourse.mybir as mybir  # noqa: E402
import concourse.tile as tile  # noqa: E402
from concourse.bass_utils import run_bass_kernel_spmd  # noqa: E402
from concourse.vector_clock import ScopedClock  # noqa: E402

BF16 = mybir.dt.bfloat16
F32 = mybir.dt.float32
NPBF16 = ml_dtypes.bfloat16

B, S, E, H, D, K = 1, 4096, 2048, 16, 128, 8
TILE = 128
T = S // TILE          # 32 query/key tiles
NCORES = 8
HPC = H // NCORES      # heads per core = 2
CHUNK = 512            # s-chunk for projections
NCHUNK = S // CHUNK
SM_SCALE = 1.0 / math.sqrt(D)
NTRI = T * (T + 1) // 2  # 528 (t,v<=t) pairs per head

_PATCHED = False


def _patch_tile_drain():
    """This container's walrus caps per-instruction sync waits; the Tile
    kernel-tail drain carries one wait per live semaphore. Split them onto
    preceding SP nops."""
    global _PATCHED
    if _PATCHED:
        return
    _PATCHED = True

    def _drain_and_barrier(self, tick_clock, wait_clock):
        nc = self.nc
        nops = []
        nsems = len(self.sems.allocated()) if self.sems is not None else 0
        for _ in range(nsems):
            nops.append(nc.sync.nop())
        drain_inst = nc.sync.drain()
        wait_clock.add_sem_waits(
            drain_inst.ins, ScopedClock({None: tick_clock.global_clock})
        )
        si = drain_inst.ins.sync_info
        waits = list(si.on_wait or [])
        if len(waits) > 1:
            si.on_wait = waits[:1]
            for i, w in enumerate(waits[1:]):
                ni = nops[i].ins if hasattr(nops[i], "ins") else nops[i]
                nsi = ni.sync_info
                if nsi is None:
                    ni.sync_info = mybir.SyncInfo(on_wait=[w], on_update=[])
                else:
                    nsi.on_wait = [w]
        nc.all_engine_barrier()
        assert self.sems is not None
        popped = nc._tile_sem_poison_stack.pop()
        assert popped is self._sem_poison
        nc.clear_and_free_semaphores(list(self.sems.allocated().values()))
        nc.all_engine_barrier()
        _split_multi_waits(nc)

    tile.TileContext._drain_and_barrier = _drain_and_barrier


def _split_multi_waits(nc):
    """Walrus here encodes at most one sync-wait per instruction; move the
    extras onto preceding same-engine no-ops."""
    ctr = [0]
    for f in nc.m.functions:
        for bb in f.blocks:
            insts = list(bb.instructions)
            if not any(
                i.sync_info and i.sync_info.on_wait
                and len(i.sync_info.on_wait) > 1
                for i in insts
            ):
                continue
            newl = []
            for inst in insts:
                si = inst.sync_info
                if si and si.on_wait and len(si.on_wait) > 1:
                    waits = list(si.on_wait)
                    for w in waits[:-1]:
                        ctr[0] += 1
                        nop = mybir.InstNoOp(
                            name=f"WSPL-{ctr[0]}", ins=[], outs=[])
                        nop.engine = inst.engine
                        nop.sync_info = mybir.SyncInfo(
                            on_wait=[w], on_update=[])
                        newl.append(nop)
                    si.on_wait = waits[-1:]
                newl.append(inst)
            bb.instructions = newl


def _tri_col(t, v):
    return t * (t + 1) // 2 + v


def build_bass():
    """Uniform per-core program. Inputs (per core, bf16 unless noted):
    xT [E, S], wqk [E, 4*128] (q_h0,q_h1,k_h0,k_h1), wv [E, 256],
    wo [256, E], cosT/sinT [128, S], rotT [128,128] (R^T for rotate_half),
    triT [128,128] (tri[l,q] = l<=q), idT [128,128] identity,
    mw [128, HPC*NTRI] f32 weight columns.
    Output: outT [E, S] bf16 (partial x@.. contribution of this core's heads).
    """
    nc = bass.Bass()
    xT = nc.dram_tensor("xT", [E, S], BF16, kind="ExternalInput")
    wqk = nc.dram_tensor("wqk", [E, 4 * TILE], BF16, kind="ExternalInput")
    wv = nc.dram_tensor("wv", [E, 2 * TILE], BF16, kind="ExternalInput")
    wo = nc.dram_tensor("wo", [2 * TILE, E], BF16, kind="ExternalInput")
    cosT = nc.dram_tensor("cosT", [TILE, S], BF16, kind="ExternalInput")
    sinT = nc.dram_tensor("sinT", [TILE, S], BF16, kind="ExternalInput")
    rotT = nc.dram_tensor("rotT", [TILE, TILE], BF16, kind="ExternalInput")
    triT = nc.dram_tensor("triT", [TILE, TILE], BF16, kind="ExternalInput")
    idT = nc.dram_tensor("idT", [TILE, TILE], BF16, kind="ExternalInput")
    mw = nc.dram_tensor("mw", [TILE, HPC * NTRI], F32, kind="ExternalInput")
    outT = nc.dram_tensor("outT", [E, S], BF16, kind="ExternalOutput")

    EK = E // TILE  # 16 contraction tiles

    with tile.TileContext(nc) as tc:
        with tc.tile_pool(name="const", bufs=1) as cpool:
            sb_wqk = cpool.tile([TILE, EK, 4 * TILE], BF16)
            sb_wv = cpool.tile([TILE, EK, 2 * TILE], BF16)
            sb_wo = cpool.tile([TILE, 2, E], BF16)
            sb_cos = cpool.tile([TILE, S], BF16)
            sb_sin = cpool.tile([TILE, S], BF16)
            sb_rot = cpool.tile([TILE, TILE], BF16)
            sb_tri = cpool.tile([TILE, TILE], BF16)
            sb_id = cpool.tile([TILE, TILE], BF16)
            sb_mw = cpool.tile([TILE, HPC * NTRI], F32)
            ones_col = cpool.tile([TILE, 1], BF16)
            # persistent per-head tensors (bf16): qT/kT [d, S], v [s-tiles, d]
            sb_q = cpool.tile([TILE, HPC, S], BF16, tag="q")
            sb_k = cpool.tile([TILE, HPC, S], BF16, tag="k")
            sb_v = cpool.tile([TILE, HPC, S], BF16, tag="v")
            sb_attn = cpool.tile([TILE, HPC, S], BF16, tag="attn")

            nc.sync.dma_start(out=sb_wqk[:],
                              in_=wqk.rearrange("(a p) b -> p a b", p=TILE))
            nc.sync.dma_start(out=sb_wv[:],
                              in_=wv.rearrange("(a p) b -> p a b", p=TILE))
            nc.sync.dma_start(out=sb_wo[:],
                              in_=wo.rearrange("(a p) b -> p a b", p=TILE))
            nc.sync.dma_start(out=sb_cos[:], in_=cosT[:])
            nc.sync.dma_start(out=sb_sin[:], in_=sinT[:])
            nc.sync.dma_start(out=sb_rot[:], in_=rotT[:])
            nc.sync.dma_start(out=sb_tri[:], in_=triT[:])
            nc.sync.dma_start(out=sb_id[:], in_=idT[:])
            nc.sync.dma_start(out=sb_mw[:], in_=mw[:])
            nc.vector.memset(ones_col[:], 1.0)

            # ---------------- Phase 1: projections + RoPE ----------------
            # Chunk pairs: for each m-tile the two chunks' accumulation
            # steps share the stationary weight tile back-to-back.
            with (
                tc.tile_pool(name="xin", bufs=2) as xpool,
                tc.tile_pool(name="ptmp", bufs=3) as tpool,
                tc.tile_pool(name="qkps", bufs=2, space="PSUM") as qkps,
                tc.tile_pool(name="vps", bufs=2, space="PSUM") as vps,
                tc.tile_pool(name="rops", bufs=2, space="PSUM") as rops,
            ):
                for cp in range(NCHUNK // 2):
                    xts = []
                    for half in range(2):
                        s0 = (2 * cp + half) * CHUNK
                        xt = xpool.tile([TILE, EK, CHUNK], BF16, tag="xt")
                        nc.sync.dma_start(
                            out=xt[:],
                            in_=xT[:, s0:s0 + CHUNK].rearrange(
                                "(a p) b -> p a b", p=TILE),
                        )
                        xts.append(xt)
                    # qT/kT M-tiles: 0=q_h0 1=q_h1 2=k_h0 3=k_h1
                    for m in range(4):
                        pss = [qkps.tile([TILE, CHUNK], F32, tag="qk",
                                         name=f"qk{m}a"),
                               qkps.tile([TILE, CHUNK], F32, tag="qk",
                                         name=f"qk{m}b")]
                        for e in range(EK):
                            for half in range(2):
                                nc.tensor.matmul(
                                    pss[half][:],
                                    sb_wqk[:, e, m * TILE:(m + 1) * TILE],
                                    xts[half][:, e, :], start=(e == 0),
                                    stop=(e == EK - 1),
                                    skip_group_check=True)
                        for half in range(2):
                            s0 = (2 * cp + half) * CHUNK
                            ps = pss[half]
                            raw = tpool.tile([TILE, CHUNK], BF16, tag="raw")
                            nc.scalar.copy(out=raw[:], in_=ps[:])
                            # rotate_half via partition-shifted copies
                            # instead of the R^T matmul: frees the PE and
                            # the rot-PSUM round trip
                            rsh = tpool.tile([TILE, CHUNK], BF16, tag="rsh")
                            nc.scalar.activation(
                                out=rsh[0:D // 2, :],
                                in_=raw[D // 2:D, :],
                                func=mybir.ActivationFunctionType.Copy,
                                scale=-1.0)
                            nc.vector.tensor_copy(rsh[D // 2:D, :],
                                                  raw[0:D // 2, :])
                            t1 = tpool.tile([TILE, CHUNK], BF16, tag="t1")
                            nc.gpsimd.tensor_mul(t1[:], raw[:],
                                                 sb_cos[:, s0:s0 + CHUNK])
                            t2 = tpool.tile([TILE, CHUNK], BF16, tag="t2")
                            nc.vector.tensor_mul(t2[:], rsh[:],
                                                 sb_sin[:, s0:s0 + CHUNK])
                            dst = sb_q if m < 2 else sb_k
                            h = m % 2
                            nc.vector.tensor_add(dst[:, h, s0:s0 + CHUNK],
                                                 t1[:], t2[:])
                    # v: M-tiles over s (8 per chunk pair), N = 2 heads * 128
                    for sm in range(2 * CHUNK // TILE):
                        vp = vps.tile([TILE, 2 * TILE], F32, tag="v")
                        st = 2 * cp * CHUNK + sm * TILE
                        xt = xts[sm // (CHUNK // TILE)]
                        lo = (sm % (CHUNK // TILE)) * TILE
                        for e in range(EK):
                            nc.tensor.matmul(
                                vp[:], xt[:, e, lo:lo + TILE],
                                sb_wv[:, e, :], start=(e == 0),
                                stop=(e == EK - 1))
                        for h in range(HPC):
                            if h == 0:
                                nc.scalar.copy(
                                    out=sb_v[:, h, st:st + TILE],
                                    in_=vp[:, h * TILE:(h + 1) * TILE])
                            else:
                                nc.vector.tensor_copy(
                                    sb_v[:, h, st:st + TILE],
                                    vp[:, h * TILE:(h + 1) * TILE])

            # ---------------- Phase 2: block-sparse attention ------------
            # Per (h, t): logits [l,q] per past tile v, exp, multiplicity
            # weight, then PV flipped to out[q, d] with wt stationary so the
            # denominator is a 1-column matmul. Normalize with a [q,1]
            # reciprocal, transpose back to [d, q] on the PE.
            GRP = 4  # logits tiles per psum bank
            with (
                tc.tile_pool(name="wt", bufs=4) as wtp,
                tc.tile_pool(name="nrm", bufs=3) as nrm,
                tc.tile_pool(name="lg", bufs=3, space="PSUM") as lgps,
                tc.tile_pool(name="ot", bufs=2, space="PSUM") as otps,
                tc.tile_pool(name="dn", bufs=2, space="PSUM") as dnps,
                tc.tile_pool(name="tp", bufs=1, space="PSUM") as tpps,
            ):
                pending = None  # (at_tile, h, t) awaiting transpose
                for h in range(HPC):
                    for t in range(T):
                        nv = t + 1
                        q_sl = sb_q[:, h, t * TILE:(t + 1) * TILE]
                        out_ps = otps.tile([TILE, TILE], F32, tag="ot")
                        den_ps = dnps.tile([TILE, 1], F32, tag="dn")
                        ngrp = (nv + GRP - 1) // GRP
                        first = True
                        for g in range(ngrp):
                            v0 = g * GRP
                            gn = min(GRP, nv - v0)
                            lg = lgps.tile([TILE, GRP * TILE], F32, tag="lg")
                            for j in range(gn):
                                v = v0 + j
                                nc.tensor.matmul(
                                    lg[:, j * TILE:(j + 1) * TILE],
                                    sb_k[:, h, v * TILE:(v + 1) * TILE],
                                    q_sl, start=True, stop=True)
                            wt = wtp.tile([TILE, GRP * TILE], BF16, tag="wt")
                            nc.scalar.activation(
                                out=wt[:, :gn * TILE], in_=lg[:, :gn * TILE],
                                func=mybir.ActivationFunctionType.Exp,
                                scale=SM_SCALE)
                            for j in range(gn):
                                v = v0 + j
                                mcol = sb_mw[:, h * NTRI + _tri_col(t, v):
                                             h * NTRI + _tri_col(t, v) + 1]
                                wsl = wt[:, j * TILE:(j + 1) * TILE]
                                if v == t:
                                    nc.vector.scalar_tensor_tensor(
                                        out=wsl, in0=wsl, scalar=mcol,
                                        in1=sb_tri[:],
                                        op0=mybir.AluOpType.mult,
                                        op1=mybir.AluOpType.mult)
                                else:
                                    nc.vector.tensor_scalar_mul(wsl, wsl,
                                                                mcol)
                            for j in range(gn):
                                v = v0 + j
                                last = (g == ngrp - 1) and (j == gn - 1)
                                wsl = wt[:, j * TILE:(j + 1) * TILE]
                                # out[q, d] += wt^T-contracted v tile;
                                # den[q, 1] shares wt as stationary.
                                nc.tensor.matmul(
                                    out_ps[:], wsl,
                                    sb_v[:, h, v * TILE:(v + 1) * TILE],
                                    start=first, stop=last,
                                    skip_group_check=True)
                                nc.tensor.matmul(
                                    den_ps[:], wsl, ones_col[:],
                                    start=first, stop=last,
                                    skip_group_check=True)
                                first = False
                        # normalize: per-partition recip (DVE), scale on ACT
                        rc = nrm.tile([TILE, 1], F32, tag="rc")
                        nc.vector.reciprocal(out=rc[:], in_=den_ps[:])
                        at = nrm.tile([TILE, TILE], BF16, tag="at")
                        nc.scalar.activation(
                            out=at[:], in_=out_ps[:],
                            func=mybir.ActivationFunctionType.Copy,
                            scale=rc[:])
                        # transpose back to [d, q] one iteration late so the
                        # PE never stalls on the recip/normalize chain
                        if pending is not None:
                            pat, ph, pt = pending
                            tp = tpps.tile([TILE, TILE], BF16, tag="tp")
                            nc.tensor.transpose(tp[:], pat[:], sb_id[:])
                            nc.scalar.copy(
                                out=sb_attn[:, ph, pt * TILE:(pt + 1) * TILE],
                                in_=tp[:])
                        pending = (at, h, t)
                if pending is not None:
                    pat, ph, pt = pending
                    tp = tpps.tile([TILE, TILE], BF16, tag="tp")
                    nc.tensor.transpose(tp[:], pat[:], sb_id[:])
                    nc.scalar.copy(
                        out=sb_attn[:, ph, pt * TILE:(pt + 1) * TILE],
                        in_=tp[:])

            # ---------------- Phase 3: output projection -----------------
            with (
                tc.tile_pool(name="ost", bufs=3) as ost,
                tc.tile_pool(name="wops", bufs=3, space="PSUM") as wops,
            ):
                for ci in range(NCHUNK):
                    s0 = ci * CHUNK
                    for m in range(EK):  # output e tiles
                        op = wops.tile([TILE, CHUNK], F32, tag="op")
                        for h in range(HPC):
                            nc.tensor.matmul(
                                op[:], sb_wo[:, h, m * TILE:(m + 1) * TILE],
                                sb_attn[:, h, s0:s0 + CHUNK],
                                start=(h == 0), stop=(h == HPC - 1))
                        ob = ost.tile([TILE, CHUNK], BF16, tag="ob")
                        if m % 2 == 0:
                            nc.scalar.copy(out=ob[:], in_=op[:])
                            # ACT copied this tile; SP issues its DMA
                            nc.sync.dma_start(
                                out=outT[m * TILE:(m + 1) * TILE,
                                         s0:s0 + CHUNK],
                                in_=ob[:])
                        else:
                            nc.vector.tensor_copy(ob[:], op[:])
                            # split the output-DMA issue load: the ACT
                            # engine also fronts a HWDGE queue and idles
                            # during the phase-3 drain
                            nc.scalar.dma_start(
                                out=outT[m * TILE:(m + 1) * TILE,
                                         s0:s0 + CHUNK],
                                in_=ob[:])
    return nc


def _host_prep(x, wq, wk, wv, wo, rope_angles, anchor_indices):
    xT = np.ascontiguousarray(x[0].T).astype(NPBF16)
    cos = np.cos(rope_angles.astype(np.float64))
    sin = np.sin(rope_angles.astype(np.float64))
    cosT = np.ascontiguousarray(
        np.concatenate([cos, cos], axis=1).T).astype(NPBF16)
    sinT = np.ascontiguousarray(
        np.concatenate([sin, sin], axis=1).T).astype(NPBF16)
    half = D // 2
    R = np.zeros((D, D), np.float32)
    for d in range(half):
        R[d, d + half] = -1.0
        R[d + half, d] = 1.0
    rotT = np.ascontiguousarray(R.T).astype(NPBF16)
    tri = (np.arange(TILE)[:, None] <= np.arange(TILE)[None, :])
    triT = tri.astype(NPBF16)
    idT = np.eye(TILE, dtype=NPBF16)

    in_maps = []
    for c in range(NCORES):
        heads = [c * HPC + i for i in range(HPC)]
        wqk_c = np.concatenate(
            [wq[:, h * D:(h + 1) * D] for h in heads]
            + [wk[:, h * D:(h + 1) * D] for h in heads], axis=1)
        wv_c = np.concatenate([wv[:, h * D:(h + 1) * D] for h in heads],
                              axis=1)
        wo_c = np.concatenate([wo[h * D:(h + 1) * D, :] for h in heads],
                              axis=0)
        mwc = np.zeros((TILE, HPC * NTRI), np.float32)
        for i, h in enumerate(heads):
            for t in range(T):
                sel = list(anchor_indices[0, h, t]) + [t]
                for v in range(t + 1):
                    m = sel.count(v)
                    if m:
                        mwc[:, i * NTRI + _tri_col(t, v)] = float(m)
        in_maps.append({
            "xT": xT, "wqk": np.ascontiguousarray(wqk_c).astype(NPBF16),
            "wv": np.ascontiguousarray(wv_c).astype(NPBF16),
            "wo": np.ascontiguousarray(wo_c).astype(NPBF16),
            "cosT": cosT, "sinT": sinT, "rotT": rotT, "triT": triT,
            "idT": idT, "mw": mwc,
        })
    return in_maps


def kernel(x, wq, wk, wv, wo, rope_angles, anchor_indices, **run_kwargs):
    _patch_tile_drain()
    nc = build_bass()
    in_maps = _host_prep(x, wq, wk, wv, wo, rope_angles, anchor_indices)
    res = run_bass_kernel_spmd(nc, in_maps, core_ids=list(range(NCORES)),
                               **run_kwargs)
    acc = np.zeros((E, S), np.float64)
    for c in range(NCORES):
        acc += res.results[c]["outT"].astype(np.float64)
    out = np.ascontiguousarray(acc.T.reshape(B, S, E)).astype(np.float32)
    kernel.last_results = res
    return out
